# revision 4
# baseline (speedup 1.0000x reference)
"""Swin-style basic block (W-MSA + CNN-MLP) Trainium2 kernel, 8-way sharded.

v2: full-partition v-projection, window-pair-batched softmax (bigger ACT/DVE
ops), sc2/xattn kept in SBUF as bf16 (no DRAM round trip), depthwise-conv
matmuls at N=512, quad-row input DMA.

Sharding: 8 shards = (batch b in 0..3) x (top/bottom half of the 128x128
image).  Each core receives 10 window-rows of input (80 pixel rows: its own
64 plus one full window-row of halo above and below, zero-padded outside the
image).
"""

import numpy as np
import ml_dtypes
from contextlib import ExitStack

B_, HI, WI, C = 4, 128, 128, 256
WS, NH, HD = 8, 8, 32
HID = 1024
BN_EPS = 1e-5
NCORES = 8
NWR = 10            # window-rows per core (8 own + 2 halo)
AROWS = 8 * NWR     # 80
KROWS = 66          # kept x_attn rows: local pixel rows 7..73
OROWS = 64
TA = AROWS * WI     # 10240
TK = KROWS * WI     # 8448
TO = OROWS * WI     # 8192

BF16 = ml_dtypes.bfloat16

# depthwise-conv engine split by 128-channel subgroup b (0..7)
DW_PE = (0, 1, 2)            # diagonal matmuls on TensorE
DW_POOL = (3,)               # mult+add tensor_tensor chain on GpSimd
DW_DVE = (4, 5, 6, 7)        # fused MAC (scalar_tensor_tensor) on VectorE

_BUILD_CACHE = {}
LAST_RESULTS = None


def _sub_ap(base, part0, nparts, free_off, free_dims):
    import concourse.bass as bass
    pstride = base.ap[0][0]
    return bass.AP(
        tensor=base.tensor,
        offset=base.offset + part0 * pstride + free_off,
        ap=[[pstride, nparts]] + [list(d) for d in free_dims],
    )


def _build(flags):
    import concourse.bass as bass
    import concourse.tile as tile
    from concourse import bacc, mybir
    from concourse.masks import make_identity

    qk_bias_nz, v_bias_nz, dn_bias_nz, sc2_affine = flags
    f32 = mybir.dt.float32
    bf = mybir.dt.bfloat16
    ALU = mybir.AluOpType
    ACTF = mybir.ActivationFunctionType
    AX = mybir.AxisListType

    nc = bacc.Bacc("TRN2", target_bir_lowering=False, debug=False,
                   num_devices=NCORES)

    # ---------------- DRAM tensors ----------------
    xs_d = nc.dram_tensor("xs", [TA, C], f32, kind="ExternalInput")
    wqk_d = nc.dram_tensor("wqk", [C, 2 * C], bf, kind="ExternalInput")
    wv_d = nc.dram_tensor("wv", [C, C], bf, kind="ExternalInput")
    wproj_d = nc.dram_tensor("wproj", [C, C], bf, kind="ExternalInput")
    expb_d = nc.dram_tensor("expb", [128, 512], bf, kind="ExternalInput")
    wup_d = nc.dram_tensor("wup", [C, HID], bf, kind="ExternalInput")
    wdn_d = nc.dram_tensor("wdn", [HID, C], bf, kind="ExternalInput")
    dwdiag_d = nc.dram_tensor("dwdiag", [128, 9 * 8 * 32], bf, kind="ExternalInput")
    dwcol_d = nc.dram_tensor("dwcol", [128, 8 * 9], f32, kind="ExternalInput")
    dwfull_d = nc.dram_tensor("dwfull", [128, len(DW_POOL) * 9 * 512], bf,
                              kind="ExternalInput")
    upb_d = nc.dram_tensor("upb", [128, 8], f32, kind="ExternalInput")
    bnb_d = nc.dram_tensor("bnb", [128, 8], f32, kind="ExternalInput")
    qkb_d = nc.dram_tensor("qkb", [128, 4], f32, kind="ExternalInput")
    vb_d = nc.dram_tensor("vbr", [128, C], f32, kind="ExternalInput")
    dnb_d = nc.dram_tensor("dnb", [128, 2], f32, kind="ExternalInput")
    g2r_d = nc.dram_tensor("g2r", [128, C], f32, kind="ExternalInput")
    b2r_d = nc.dram_tensor("b2r", [128, C], f32, kind="ExternalInput")

    out_d = nc.dram_tensor("out", [TO, C], f32, kind="ExternalOutput")

    with tile.TileContext(nc) as tc, ExitStack() as octx:
        consts = octx.enter_context(tc.tile_pool(name="consts", bufs=1))
        persist = octx.enter_context(tc.tile_pool(name="persist", bufs=1))

        eps_sb = consts.tile([128, 1], f32)
        nc.vector.memset(eps_sb[:], 1e-5)
        ident = consts.tile([128, 128], bf)
        make_identity(nc, ident[:])

        wqk_sb = consts.tile([128, 2, 2 * C], bf)
        nc.sync.dma_start(wqk_sb[:], wqk_d[:, :].rearrange("(k p) o -> p k o", k=2))
        wv_sb = consts.tile([128, 2, C], bf)
        nc.sync.dma_start(wv_sb[:], wv_d[:, :].rearrange("(k p) o -> p k o", k=2))
        wproj_sb = consts.tile([128, 2, C], bf)
        nc.sync.dma_start(wproj_sb[:], wproj_d[:, :].rearrange("(k p) o -> p k o", k=2))
        expb_sb = consts.tile([128, 512], bf)
        nc.sync.dma_start(expb_sb[:], expb_d[:, :])
        wup_sb = consts.tile([128, 2, HID], bf)
        nc.sync.dma_start(wup_sb[:], wup_d[:, :].rearrange("(k p) o -> p k o", k=2))
        wdn_sb = consts.tile([128, 8, C], bf)
        nc.sync.dma_start(wdn_sb[:], wdn_d[:, :].rearrange("(k p) o -> p k o", k=8))
        dwdiag_sb = consts.tile([128, 9 * 8 * 32], bf)
        nc.sync.dma_start(dwdiag_sb[:], dwdiag_d[:, :])
        dwcol_sb = consts.tile([128, 8 * 9], f32)
        nc.sync.dma_start(dwcol_sb[:], dwcol_d[:, :])
        dwfull_sb = consts.tile([128, len(DW_POOL) * 9 * 512], bf)
        nc.sync.dma_start(dwfull_sb[:], dwfull_d[:, :])
        upb_sb = consts.tile([128, 8], f32)
        nc.sync.dma_start(upb_sb[:], upb_d[:, :])
        bnb_sb = consts.tile([128, 8], f32)
        nc.sync.dma_start(bnb_sb[:], bnb_d[:, :])
        qkb_sb = consts.tile([128, 4], f32)
        nc.sync.dma_start(qkb_sb[:], qkb_d[:, :])
        vb_sb = consts.tile([128, C], f32)
        nc.sync.dma_start(vb_sb[:], vb_d[:, :])
        dnb_sb = consts.tile([128, 2], f32)
        nc.sync.dma_start(dnb_sb[:], dnb_d[:, :])
        g2r_sb = consts.tile([128, C], f32)
        b2r_sb = consts.tile([128, C], f32)
        if sc2_affine:
            nc.sync.dma_start(g2r_sb[:], g2r_d[:, :])
            nc.sync.dma_start(b2r_sb[:], b2r_d[:, :])

        xn2T = [persist.tile([128, TK], bf, tag=f"xn2T{ct}", name=f"xn2T{ct}") for ct in range(2)]
        # row-major persistent store of LN2 output: [128 x-parts, KROWS*C]
        sc2_sb = persist.tile([128, KROWS * C], bf, tag="sc2sb", name="sc2sb")
        # per-pixel LN2 stats for xattn reconstruction: xattn = xn2*sigma + m
        # (sg_sb holds sigma+1 when not affine, sigma when affine)
        m_sb = persist.tile([128, AROWS], f32, tag="m_sb", name="m_sb")
        sg_sb = persist.tile([128, AROWS], f32, tag="sg_sb", name="sg_sb")

        # ======================= STAGE A =======================
        with ExitStack() as actx:
            xrow_p = actx.enter_context(tc.tile_pool(name="xrow", bufs=4))
            ln_p = actx.enter_context(tc.tile_pool(name="ln", bufs=4))
            mv_p = actx.enter_context(tc.tile_pool(name="mv", bufs=4))
            xnT_p = actx.enter_context(tc.tile_pool(name="xnT", bufs=2))
            qk_p = actx.enter_context(tc.tile_pool(name="qk", bufs=2))
            v_p = actx.enter_context(tc.tile_pool(name="vp", bufs=10))
            at_p = actx.enter_context(tc.tile_pool(name="at", bufs=2))
            small_p = actx.enter_context(tc.tile_pool(name="small", bufs=6))
            proj_p = actx.enter_context(tc.tile_pool(name="proj", bufs=2))
            oT_p = actx.enter_context(tc.tile_pool(name="oTp", bufs=9))
            xat_p = actx.enter_context(tc.tile_pool(name="xatp", bufs=9))
            tail_p = actx.enter_context(tc.tile_pool(name="tail", bufs=3))
            ps_tp = actx.enter_context(tc.tile_pool(name="ps_tp", bufs=2, space="PSUM"))
            ps_mm = actx.enter_context(tc.tile_pool(name="ps_mm", bufs=2, space="PSUM"))
            ps_sc = actx.enter_context(tc.tile_pool(name="ps_sc", bufs=2, space="PSUM"))
            ps_pat = actx.enter_context(tc.tile_pool(name="ps_pat", bufs=1, space="PSUM"))
            ps_oT = actx.enter_context(tc.tile_pool(name="ps_oT", bufs=1, space="PSUM"))

            for wr in range(NWR):
                xq = [xrow_p.tile([128, 4, C], f32, tag=f"xq{i}", name=f"xq{i}")
                      for i in range(2)]
                for h4 in range(2):
                    y0 = 8 * wr + 4 * h4
                    nc.sync.dma_start(
                        xq[h4][:],
                        xs_d[y0 * WI:(y0 + 4) * WI, :]
                        .rearrange("(r p) c -> p r c", p=WI))
                xrows = [xq[ry // 4][:, ry % 4, :] for ry in range(8)]
                xnT = [xnT_p.tile([128, 1024], bf, tag=f"xnT{ct}", name=f"xnT{ct}") for ct in range(2)]
                mv8 = mv_p.tile([128, 8, 2], f32, tag="mv8", name="mv8")
                rstd8 = mv_p.tile([128, 8], f32, tag="rstd8", name="rstd8")
                lvar8 = mv_p.tile([128, 8], f32, tag="lvar8", name="lvar8")
                for ry in range(8):
                    st = mv_p.tile([128, 6], f32, tag="st", name="st")
                    nc.vector.bn_stats(st[:], xrows[ry])
                    nc.vector.bn_aggr(mv8[:, ry, :], st[:])
                nc.scalar.activation(lvar8[:], mv8[:, :, 1], ACTF.Ln,
                                     bias=eps_sb[:], scale=1.0)
                nc.scalar.activation(rstd8[:], lvar8[:], ACTF.Exp, scale=-0.5)
                mrs8 = mv_p.tile([128, 8], f32, tag="mrs8", name="mrs8")
                nc.vector.tensor_tensor(out=mrs8[:], in0=mv8[:, :, 0],
                                        in1=rstd8[:], op=ALU.mult)
                for ry in range(8):
                    # xnb = (x - m) * rstd = x*rstd - m*rstd, fused
                    xnb = ln_p.tile([128, C], bf, tag="xnb", name="xnb")
                    nc.vector.scalar_tensor_tensor(
                        out=xnb[:], in0=xrows[ry], scalar=rstd8[:, ry:ry + 1],
                        in1=_sub_ap(mrs8[:], 0, 128, ry, [[0, C]]),
                        op0=ALU.mult, op1=ALU.subtract)
                    for ct in range(2):
                        tp = ps_tp.tile([128, 512], bf, tag="ps", name="ps")
                        nc.tensor.transpose(tp[:, 0:128],
                                            xnb[:, 128 * ct:128 * ct + 128], ident[:])
                        # scatter row-major pixel row -> window-ordered columns
                        nc.vector.tensor_copy(
                            _sub_ap(xnT[ct][:], 0, 128, 8 * ry, [[64, 16], [1, 8]]),
                            tp[:, 0:128])

                # ---- q/k projections: [oc-tile][128, 1024 tok(window-order)]
                qkT = [qk_p.tile([128, 1024], bf, tag=f"qkT{m}", name=f"qkT{m}") for m in range(4)]
                for m in range(4):
                    for j in range(2):
                        pqk = ps_mm.tile([128, 512], f32, tag="ps", name="ps")
                        for kt in range(2):
                            rhs = xnT[kt][:, 512 * j:512 * j + 512]
                            nc.tensor.matmul(pqk[:],
                                             lhsT=wqk_sb[:, kt, 128 * m:128 * m + 128],
                                             rhs=rhs, start=(kt == 0), stop=(kt == 1))
                        if qk_bias_nz:
                            nc.vector.tensor_scalar_add(
                                qkT[m][:, 512 * j:512 * j + 512], pqk[:],
                                qkb_sb[:, m:m + 1])
                        else:
                            nc.vector.tensor_copy(qkT[m][:, 512 * j:512 * j + 512],
                                                  pqk[:])

                # ---- v per window-pair: [64 tok, 512] (win-major, bf16)
                vsbs = []
                for u in range(8):
                    pv = ps_mm.tile([128, 512], f32, tag="ps", name="ps")
                    for w in range(2):
                        for kt in range(2):
                            lhsT = xnT[kt][:, 64 * (2 * u + w):64 * (2 * u + w) + 64]
                            nc.tensor.matmul(pv[0:64, 256 * w:256 * w + 256],
                                             lhsT=lhsT, rhs=wv_sb[:, kt, :],
                                             start=(kt == 0), stop=(kt == 1))
                    v2 = v_p.tile([128, 512], bf, tag="v2", name="v2")
                    if v_bias_nz:
                        nc.vector.tensor_tensor(
                            out=v2[0:64, :], in0=pv[0:64, :],
                            in1=_sub_ap(vb_sb[:], 0, 64, 0, [[0, 2], [1, C]]),
                            op=ALU.add)
                    else:
                        nc.scalar.copy(v2[0:64, :], pv[0:64, :])
                    vsbs.append(v2)

                # ---- attention per window-pair
                oTs = []
                for u in range(8):
                    pscs = [ps_sc.tile([128, 512], f32, tag="ps", name="ps") for _ in range(4)]
                    for h in range(NH):
                        g, jh = h % 4, h // 4
                        qt_t = qkT[h // 4]
                        kt_t = qkT[2 + h // 4]
                        for w in range(2):
                            tok0 = 64 * (2 * u + w)
                            nc.tensor.matmul(
                                pscs[g][64 * w:64 * w + 64, 64 * jh:64 * jh + 64],
                                lhsT=qt_t[32 * g:32 * g + 32, tok0:tok0 + 64],
                                rhs=kt_t[32 * g:32 * g + 32, tok0:tok0 + 64],
                                start=True, stop=True,
                                tile_position=(32 * g, 64 * w))
                    attn_e = at_p.tile([128, 512], bf, tag="attn_e", name="attn_e")
                    for g in range(4):
                        nc.scalar.activation(attn_e[:, 128 * g:128 * g + 128],
                                             pscs[g][:, 0:128], ACTF.Exp)
                    attn_u = at_p.tile([128, 512], bf, tag="attn_u", name="attn_u")
                    nc.gpsimd.tensor_tensor(out=attn_u[:], in0=attn_e[:],
                                            in1=expb_sb[:], op=ALU.mult)
                    r8 = small_p.tile([128, 8], f32, tag="r8", name="r8")
                    nc.vector.tensor_reduce(
                        r8[:], attn_u[:].rearrange("p (a k) -> p a k", a=8),
                        axis=AX.X, op=ALU.add)
                    rr8 = small_p.tile([128, 8], f32, tag="rr8", name="rr8")
                    nc.vector.reciprocal(rr8[:], r8[:])
                    attn_n = at_p.tile([128, 512], bf, tag="attn_n", name="attn_n")
                    nc.vector.tensor_tensor(
                        out=attn_n[:], in0=attn_u[:],
                        in1=_sub_ap(rr8[:], 0, 128, 0, [[1, 8], [0, 64]]),
                        op=ALU.mult)
                    aT = []
                    for g in range(4):
                        pat = ps_pat.tile([128, 512], bf, tag="ps", name="ps")
                        for jh in range(2):
                            nc.tensor.transpose(
                                pat[0:64, 128 * jh:128 * jh + 128],
                                attn_n[:, 128 * g + 64 * jh:128 * g + 64 * jh + 64],
                                ident[:])
                        t = at_p.tile([128, 256], bf, tag=f"aT{g}", name=f"aT{g}")
                        if g >= 2:
                            nc.scalar.copy(t[0:64, :], pat[0:64, 0:256])
                        else:
                            nc.vector.tensor_copy(t[0:64, :], pat[0:64, 0:256])
                        aT.append(t)
                    poT = ps_oT.tile([128, 256], f32, tag="ps", name="ps")
                    for h in range(NH):
                        g, jh = h % 4, h // 4
                        for w in range(2):
                            nc.tensor.matmul(
                                poT[32 * g:32 * g + 32,
                                    128 * jh + 64 * w:128 * jh + 64 * w + 64],
                                lhsT=vsbs[u][0:64,
                                             256 * w + 32 * h:256 * w + 32 * h + 32],
                                rhs=aT[g][0:64, 128 * jh + 64 * w:128 * jh + 64 * w + 64],
                                start=True, stop=True,
                                tile_position=(0, 32 * g))
                    ot = oT_p.tile([128, 256], bf, tag="oT", name="oT")
                    nc.scalar.copy(ot[:], poT[:])
                    oTs.append(ot)

                # ---- proj: projT [oc-tile][128, 1024] window-order
                projT = [proj_p.tile([128, 1024], bf, tag=f"projT{m}", name=f"projT{m}") for m in range(2)]
                for m in range(2):
                    for jc in range(2):
                        pp = ps_mm.tile([128, 512], f32, tag="ps", name="ps")
                        for uu in range(4):
                            u = 4 * jc + uu
                            for kt in range(2):
                                nc.tensor.matmul(
                                    pp[:, 128 * uu:128 * uu + 128],
                                    lhsT=wproj_sb[:, kt, 128 * m:128 * m + 128],
                                    rhs=oTs[u][:, 128 * kt:128 * kt + 128],
                                    start=(kt == 0), stop=(kt == 1))
                        for w in range(2):
                            # psum cols (uu, w fixed, iy, ix) -> row-major
                            src = _sub_ap(pp[:], 0, 128, 64 * w,
                                          [[128, 4], [8, 8], [1, 8]])
                            dst = _sub_ap(projT[m][:], 0, 128,
                                          8 * (8 * jc + w), [[16, 4], [128, 8], [1, 8]])
                            nc.vector.tensor_copy(dst, src)

                # ---- tail: per kept row
                if wr == 0:
                    keep = [7]
                elif wr == NWR - 1:
                    keep = [0]
                else:
                    keep = list(range(8))
                mvk = mv_p.tile([128, 8, 2], f32, tag="mvk", name="mvk")
                if wr in (0, NWR - 1):
                    nc.vector.memset(mvk[:], 0.0)
                lvk = mv_p.tile([128, 8], f32, tag="lvk", name="lvk")
                rsk = mv_p.tile([128, 8], f32, tag="rsk", name="rsk")
                xat_tiles = {}
                for ry in keep:
                    kk = 8 * wr + ry - 7
                    pfin = ps_tp.tile([128, 1024], bf, tag="ps", name="ps")
                    for m in range(2):
                        nc.tensor.transpose(pfin[:, 128 * m:128 * m + 128],
                                            projT[m][:, 128 * ry:128 * ry + 128],
                                            ident[:])
                    xat = xat_p.tile([128, C], f32, tag="xat", name="xat")
                    nc.vector.tensor_tensor(out=xat[:], in0=pfin[:, 0:256],
                                            in1=xrows[ry], op=ALU.add)
                    st2 = mv_p.tile([128, 6], f32, tag="st2", name="st2")
                    nc.vector.bn_stats(st2[:], xat[:])
                    nc.vector.bn_aggr(mvk[:, ry, :], st2[:])
                    xat_tiles[ry] = xat
                nc.scalar.activation(lvk[:], mvk[:, :, 1], ACTF.Ln,
                                     bias=eps_sb[:], scale=1.0)
                nc.scalar.activation(rsk[:], lvk[:], ACTF.Exp, scale=-0.5)
                k0, k1 = keep[0], keep[-1] + 1
                sig8 = mv_p.tile([128, 8], f32, tag="sig8", name="sig8")
                nc.scalar.activation(sig8[:, k0:k1], lvk[:, k0:k1], ACTF.Exp,
                                     scale=0.5)
                nc.vector.tensor_copy(
                    m_sb[:, 8 * wr + k0:8 * wr + k1],
                    mvk[:, k0:k1, 0])
                nc.vector.tensor_scalar_add(
                    sg_sb[:, 8 * wr + k0:8 * wr + k1], sig8[:, k0:k1],
                    0.0 if sc2_affine else 1.0)
                mrsk = mv_p.tile([128, 8], f32, tag="mrsk", name="mrsk")
                nc.vector.tensor_tensor(out=mrsk[:], in0=mvk[:, :, 0],
                                        in1=rsk[:], op=ALU.mult)
                for ry in keep:
                    kk = 8 * wr + ry - 7
                    xat = xat_tiles[ry]
                    # sc2_sb always holds the PRE-affine LN2 output (bf16)
                    xn2s = _sub_ap(sc2_sb[:], 0, 128, kk * C, [[1, C]])
                    nc.vector.scalar_tensor_tensor(
                        out=xn2s, in0=xat[:], scalar=rsk[:, ry:ry + 1],
                        in1=_sub_ap(mrsk[:], 0, 128, ry, [[0, C]]),
                        op0=ALU.mult, op1=ALU.subtract)
                    for ct in range(2):
                        tp2 = ps_tp.tile([128, 512], bf, tag="ps", name="ps2")
                        nc.tensor.transpose(
                            tp2[:, 0:128],
                            _sub_ap(sc2_sb[:], 0, 128, kk * C + 128 * ct, [[1, 128]]),
                            ident[:])
                        nc.vector.tensor_copy(xn2T[ct][:, 128 * kk:128 * kk + 128],
                                              tp2[:, 0:128])

        # ======================= STAGE B =======================
        UW = 130
        with ExitStack() as bctx:
            u_p = bctx.enter_context(tc.tile_pool(name="u_p", bufs=9))
            r_p = bctx.enter_context(tc.tile_pool(name="r_p", bufs=9))
            dnsb_p = bctx.enter_context(tc.tile_pool(name="dnsb", bufs=3))
            fin_p = bctx.enter_context(tc.tile_pool(name="fin", bufs=2))
            psU = bctx.enter_context(tc.tile_pool(name="psU", bufs=2, space="PSUM"))
            psR = bctx.enter_context(tc.tile_pool(name="psR", bufs=2, space="PSUM"))
            psD = bctx.enter_context(tc.tile_pool(name="psD", bufs=2, space="PSUM"))
            psF = bctx.enter_context(tc.tile_pool(name="psF", bufs=2, space="PSUM"))

            for q in range(4):
                kk0 = 16 * q
                Us = []
                for b in range(8):
                    U = u_p.tile([128, 18 * UW], bf, tag="U", name="U")
                    nc.vector.memset(
                        _sub_ap(U[:], 0, 128, 0, [[UW, 18], [129, 2]]), 0.0)
                    for i0 in range(0, 18, 4):
                        nrow = min(4, 18 - i0)
                        pu = psU.tile([128, 512], f32, tag="ps", name="ps")
                        for kt in range(2):
                            nc.tensor.matmul(
                                pu[:, 0:128 * nrow],
                                lhsT=wup_sb[:, kt, 128 * b:128 * b + 128],
                                rhs=xn2T[kt][:, (kk0 + i0) * WI:(kk0 + i0 + nrow) * WI],
                                start=(kt == 0), stop=(kt == 1))
                        nc.scalar.activation(
                            _sub_ap(U[:], 0, 128, i0 * UW + 1, [[UW, nrow], [1, 128]]),
                            pu[:, 0:128 * nrow], ACTF.Relu, bias=upb_sb[:, b:b + 1])
                    Us.append(U)
                for s4 in range(4):
                    jj0 = 4 * s4
                    Rs = []
                    for b in range(8):
                        R = r_p.tile([128, 512], bf, tag="R", name="R")
                        if b in DW_PE:
                            pr = psR.tile([128, 512], f32, tag="ps", name="ps")
                            for s in range(4):
                                for t in range(9):
                                    dy, dx = t // 3, t % 3
                                    lhsT = dwdiag_sb[
                                        32 * s:32 * s + 32,
                                        (t * 8 + b) * 32:(t * 8 + b) * 32 + 32]
                                    rhs = _sub_ap(Us[b][:], 32 * s, 32,
                                                  (jj0 + dy) * UW + dx,
                                                  [[UW, 4], [1, 128]])
                                    nc.tensor.matmul(
                                        pr[32 * s:32 * s + 32, 0:512],
                                        lhsT=lhsT, rhs=rhs,
                                        start=(t == 0), stop=(t == 8),
                                        tile_position=(32 * s, 32 * s))
                            nc.scalar.activation(R[:], pr[:], ACTF.Relu,
                                                 bias=bnb_sb[:, b:b + 1])
                        elif b in DW_DVE:
                            acc = [r_p.tile([128, 512], bf, tag=f"acc{i}", bufs=2,
                                            name=f"acc{i}") for i in range(2)]
                            for t in range(9):
                                dy, dx = t // 3, t % 3
                                u_in = _sub_ap(Us[b][:], 0, 128,
                                               (jj0 + dy) * UW + dx,
                                               [[UW, 4], [1, 128]])
                                wcol = dwcol_sb[:, 9 * b + t:9 * b + t + 1]
                                if t == 0:
                                    nc.vector.tensor_scalar_mul(acc[0][:], u_in, wcol)
                                else:
                                    nc.vector.scalar_tensor_tensor(
                                        out=acc[t % 2][:], in0=u_in,
                                        scalar=wcol, in1=acc[(t + 1) % 2][:],
                                        op0=ALU.mult, op1=ALU.add)
                            nc.scalar.activation(R[:], acc[0][:], ACTF.Relu,
                                                 bias=bnb_sb[:, b:b + 1])
                        else:
                            bp = DW_POOL.index(b)
                            acc = [r_p.tile([128, 512], bf, tag=f"pac{i}", bufs=2,
                                            name=f"pac{i}") for i in range(3)]
                            for t in range(9):
                                dy, dx = t // 3, t % 3
                                u_in = _sub_ap(Us[b][:], 0, 128,
                                               (jj0 + dy) * UW + dx,
                                               [[UW, 4], [1, 128]])
                                wfull = dwfull_sb[:, (9 * bp + t) * 512:
                                                  (9 * bp + t) * 512 + 512]
                                if t == 0:
                                    nc.gpsimd.tensor_tensor(
                                        out=acc[0][:], in0=u_in, in1=wfull,
                                        op=ALU.mult)
                                else:
                                    tmp = acc[2]
                                    nc.gpsimd.tensor_tensor(
                                        out=tmp[:], in0=u_in, in1=wfull,
                                        op=ALU.mult)
                                    nc.gpsimd.tensor_tensor(
                                        out=acc[t % 2][:], in0=tmp[:],
                                        in1=acc[(t + 1) % 2][:], op=ALU.add)
                            nc.scalar.activation(R[:], acc[0][:], ACTF.Relu,
                                                 bias=bnb_sb[:, b:b + 1])
                        Rs.append(R)
                    dn = dnsb_p.tile([128, 2, 512], bf, tag="dn", name="dn")
                    for m in range(2):
                        pd = psD.tile([128, 512], f32, tag="ps", name="ps")
                        for b in range(8):
                            nc.tensor.matmul(pd[:],
                                             lhsT=wdn_sb[:, b, 128 * m:128 * m + 128],
                                             rhs=Rs[b][:], start=(b == 0),
                                             stop=(b == 7))
                        if dn_bias_nz:
                            nc.vector.tensor_scalar_add(dn[:, m, :], pd[:],
                                                        dnb_sb[:, m:m + 1])
                        else:
                            nc.scalar.copy(dn[:, m, :], pd[:])
                    pf = psF.tile([128, 1024], bf, tag="pf", name="pf")
                    for jj in range(4):
                        for m in range(2):
                            nc.tensor.transpose(
                                pf[:, 256 * jj + 128 * m:256 * jj + 128 * m + 128],
                                dn[:, m, 128 * jj:128 * jj + 128], ident[:])
                    j0 = 16 * q + jj0
                    kkf = j0 + 1
                    y0 = kkf + 7   # global pixel row of first output row
                    sc4 = _sub_ap(sc2_sb[:], 0, 128, kkf * C, [[1, 4 * C]])
                    # a2 = xn2*(sigma+1) + m   (== sc + xattn when not affine)
                    a1 = fin_p.tile([128, 1024], f32, tag="a1", name="a1")
                    nc.gpsimd.tensor_tensor(
                        out=a1[:], in0=sc4,
                        in1=_sub_ap(sg_sb[:], 0, 128, y0, [[1, 4], [0, C]]),
                        op=ALU.mult)
                    a2 = fin_p.tile([128, 1024], f32, tag="a2", name="a2")
                    nc.gpsimd.tensor_tensor(
                        out=a2[:], in0=a1[:],
                        in1=_sub_ap(m_sb[:], 0, 128, y0, [[1, 4], [0, C]]),
                        op=ALU.add)
                    if sc2_affine:
                        # a2 = xattn only; shortcut = xn2*g2 + b2 added here
                        a3 = fin_p.tile([128, 1024], f32, tag="a3", name="a3")
                        nc.gpsimd.tensor_tensor(
                            out=a3[:], in0=sc4,
                            in1=_sub_ap(g2r_sb[:], 0, 128, 0, [[0, 4], [1, C]]),
                            op=ALU.mult)
                        a4 = fin_p.tile([128, 1024], f32, tag="a4", name="a4")
                        nc.gpsimd.tensor_tensor(
                            out=a4[:], in0=a3[:],
                            in1=_sub_ap(b2r_sb[:], 0, 128, 0, [[0, 4], [1, C]]),
                            op=ALU.add)
                        a5 = fin_p.tile([128, 1024], f32, tag="a5", name="a5")
                        nc.vector.tensor_tensor(out=a5[:], in0=a2[:], in1=a4[:],
                                                op=ALU.add)
                        a2 = a5
                    out4 = fin_p.tile([128, 1024], f32, tag="out4", name="out4")
                    nc.vector.tensor_tensor(
                        out=out4[:], in0=a2[:], in1=pf[:], op=ALU.add)
                    nc.sync.dma_start(
                        out_d[j0 * WI:(j0 + 4) * WI, :]
                        .rearrange("(r p) c -> p r c", p=WI),
                        out4[:].rearrange("p (r c) -> p r c", r=4))

    nc.compile()
    return nc


def _prep(g1, b1, qkv_w, qkv_b, rpb_table, rel_idx, proj_w, g2, b2,
          up_w, up_b, dw_w, bn_g, bn_b, down_w, down_b):
    f = np.float32
    g1 = np.asarray(g1, f); b1 = np.asarray(b1, f)
    qkv_w = np.asarray(qkv_w, f); qkv_b = np.asarray(qkv_b, f)
    rpb = np.asarray(rpb_table, f); ridx = np.asarray(rel_idx)
    proj_w = np.asarray(proj_w, f)
    g2 = np.asarray(g2, f); b2 = np.asarray(b2, f)
    up_w = np.asarray(up_w, f); up_b = np.asarray(up_b, f)
    dw_w = np.asarray(dw_w, f); bn_g = np.asarray(bn_g, f)
    bn_b = np.asarray(bn_b, f)
    down_w = np.asarray(down_w, f); down_b = np.asarray(down_b, f)

    sc = HD ** -0.5
    wq = qkv_w[:C] * g1[None, :] * sc
    wk = qkv_w[C:2 * C] * g1[None, :]
    wv = qkv_w[2 * C:] * g1[None, :]
    bq = (qkv_b[:C] + qkv_w[:C] @ b1) * sc
    bk = qkv_b[C:2 * C] + qkv_w[C:2 * C] @ b1
    bv = qkv_b[2 * C:] + qkv_w[2 * C:] @ b1

    wqk = np.concatenate([wq, wk], 0).T.astype(BF16).copy()
    wv_t = wv.T.astype(BF16).copy()
    wproj = proj_w.T.astype(BF16).copy()

    bias = rpb[np.asarray(ridx).reshape(-1)].reshape(64, 64, NH).transpose(2, 0, 1)
    expb = np.zeros((128, 512), f)
    for h in range(NH):
        cc = 128 * (h % 4) + 64 * (h // 4)
        eb = np.exp(bias[h])
        expb[0:64, cc:cc + 64] = eb
        expb[64:128, cc:cc + 64] = eb
    expb = expb.astype(BF16)

    wup = (up_w * g2[None, :]).T.astype(BF16).copy()
    upb = (up_b + up_w @ b2).astype(f)
    bns = bn_g * (1.0 + BN_EPS) ** -0.5
    dww = dw_w.reshape(HID, 9) * bns[:, None]
    dwdiag = np.zeros((128, 9 * 8 * 32), f)
    pp = np.arange(128)
    for b in range(8):
        for t in range(9):
            dwdiag[pp, (t * 8 + b) * 32 + (pp % 32)] = dww[128 * b + pp, t]
    dwdiag = dwdiag.astype(BF16)
    # per-partition tap weights for the DVE/Pool MAC path: dwcol[p, 9b+t]
    dwcol = np.zeros((128, 8 * 9), f)
    for b in range(8):
        dwcol[:, 9 * b:9 * b + 9] = dww[128 * b:128 * (b + 1), :]
    # free-dim-replicated tap weights for the Pool tensor_tensor path
    dwfull = np.zeros((128, len(DW_POOL) * 9 * 512), f)
    for bp, b in enumerate(DW_POOL):
        for t in range(9):
            dwfull[:, (9 * bp + t) * 512:(9 * bp + t + 1) * 512] = \
                dww[128 * b:128 * (b + 1), t:t + 1]
    dwfull = dwfull.astype(BF16)
    wdn = down_w.T.astype(BF16).copy()

    def col_n(v, n):
        return np.asarray(v, f).reshape(n, 128).T.copy()

    qkb = col_n(np.concatenate([bq, bk]), 4)
    vbr = np.broadcast_to(bv[None, :], (128, C)).astype(f).copy()
    dnb = col_n(down_b, 2)
    g2r = np.broadcast_to(g2[None, :], (128, C)).astype(f).copy()
    b2r = np.broadcast_to(b2[None, :], (128, C)).astype(f).copy()

    flags = (bool(np.any(qkb)), bool(np.any(bv)), bool(np.any(down_b)),
             not (np.allclose(g2, 1.0) and np.allclose(b2, 0.0)))

    consts = dict(wqk=wqk, wv=wv_t, wproj=wproj, expb=expb, wup=wup, wdn=wdn,
                  dwdiag=dwdiag, dwcol=dwcol, dwfull=dwfull,
                  upb=col_n(upb, 8),
                  bnb=col_n(bn_b, 8), qkb=qkb,
                  vbr=vbr, dnb=dnb, g2r=g2r, b2r=b2r)
    return consts, flags


def kernel(x, H, W, g1, b1, qkv_w, qkv_b, rpb_table, rel_idx, proj_w,
           g2, b2, up_w, up_b, dw_w, bn_g, bn_b, down_w, down_b):
    global LAST_RESULTS
    from concourse.bass_utils import run_bass_kernel_spmd

    x = np.asarray(x, np.float32)
    consts, flags = _prep(g1, b1, qkv_w, qkv_b, rpb_table, rel_idx, proj_w,
                          g2, b2, up_w, up_b, dw_w, bn_g, bn_b, down_w, down_b)
    if flags not in _BUILD_CACHE:
        _BUILD_CACHE[flags] = _build(flags)
    nc = _BUILD_CACHE[flags]

    ximg = x.reshape(B_, HI, WI, C)
    in_maps = []
    for core in range(NCORES):
        b, top = core // 2, (core % 2 == 0)
        r0 = 0 if top else 64
        xs = np.zeros((AROWS, WI, C), np.float32)
        lo, hi = r0 - 8, r0 + 72
        slo, shi = max(lo, 0), min(hi, HI)
        xs[slo - lo:shi - lo] = ximg[b, slo:shi]
        m = {"xs": xs.reshape(TA, C)}
        m.update(consts)
        in_maps.append(m)

    res = run_bass_kernel_spmd(nc, in_maps, core_ids=list(range(NCORES)))
    LAST_RESULTS = res

    out = np.empty((B_, HI, WI, C), np.float32)
    for core in range(NCORES):
        b, top = core // 2, (core % 2 == 0)
        r0 = 0 if top else 64
        out[b, r0:r0 + 64] = res.results[core]["out"].reshape(OROWS, WI, C)
    return out.reshape(B_, HI * WI, C)


# revision 5
# speedup vs baseline: 1.0231x; 1.0231x over previous
"""Swin-style basic block (W-MSA + CNN-MLP) Trainium2 kernel, 8-way sharded.

v2: full-partition v-projection, window-pair-batched softmax (bigger ACT/DVE
ops), sc2/xattn kept in SBUF as bf16 (no DRAM round trip), depthwise-conv
matmuls at N=512, quad-row input DMA.

Sharding: 8 shards = (batch b in 0..3) x (top/bottom half of the 128x128
image).  Each core receives 10 window-rows of input (80 pixel rows: its own
64 plus one full window-row of halo above and below, zero-padded outside the
image).
"""

import numpy as np
import ml_dtypes
from contextlib import ExitStack

B_, HI, WI, C = 4, 128, 128, 256
WS, NH, HD = 8, 8, 32
HID = 1024
BN_EPS = 1e-5
NCORES = 8
NWR = 10            # window-rows per core (8 own + 2 halo)
AROWS = 8 * NWR     # 80
KROWS = 66          # kept x_attn rows: local pixel rows 7..73
OROWS = 64
TA = AROWS * WI     # 10240
TK = KROWS * WI     # 8448
TO = OROWS * WI     # 8192

BF16 = ml_dtypes.bfloat16

# depthwise-conv engine split by 128-channel subgroup b (0..7)
DW_PE = (0, 1)               # diagonal matmuls on TensorE
DW_POOL = (2, 3)             # mult+add tensor_tensor chain on GpSimd
DW_DVE = (4, 5, 6, 7)        # fused MAC (scalar_tensor_tensor) on VectorE

_BUILD_CACHE = {}
LAST_RESULTS = None


def _sub_ap(base, part0, nparts, free_off, free_dims):
    import concourse.bass as bass
    pstride = base.ap[0][0]
    return bass.AP(
        tensor=base.tensor,
        offset=base.offset + part0 * pstride + free_off,
        ap=[[pstride, nparts]] + [list(d) for d in free_dims],
    )


def _build(flags):
    import concourse.bass as bass
    import concourse.tile as tile
    from concourse import bacc, mybir
    from concourse.masks import make_identity

    qk_bias_nz, v_bias_nz, dn_bias_nz, sc2_affine = flags
    f32 = mybir.dt.float32
    bf = mybir.dt.bfloat16
    ALU = mybir.AluOpType
    ACTF = mybir.ActivationFunctionType
    AX = mybir.AxisListType

    nc = bacc.Bacc("TRN2", target_bir_lowering=False, debug=False,
                   num_devices=NCORES)

    # ---------------- DRAM tensors ----------------
    xs_d = nc.dram_tensor("xs", [TA, C], f32, kind="ExternalInput")
    wqk_d = nc.dram_tensor("wqk", [C, 2 * C], bf, kind="ExternalInput")
    wv_d = nc.dram_tensor("wv", [C, C], bf, kind="ExternalInput")
    wproj_d = nc.dram_tensor("wproj", [C, C], bf, kind="ExternalInput")
    expb_d = nc.dram_tensor("expb", [128, 512], bf, kind="ExternalInput")
    wup_d = nc.dram_tensor("wup", [C, HID], bf, kind="ExternalInput")
    wdn_d = nc.dram_tensor("wdn", [HID, C], bf, kind="ExternalInput")
    dwdiag_d = nc.dram_tensor("dwdiag", [128, 9 * 8 * 32], bf, kind="ExternalInput")
    dwcol_d = nc.dram_tensor("dwcol", [128, 8 * 9], f32, kind="ExternalInput")
    dwfull_d = nc.dram_tensor("dwfull", [128, len(DW_POOL) * 9 * 512], bf,
                              kind="ExternalInput")
    upb_d = nc.dram_tensor("upb", [128, 8], f32, kind="ExternalInput")
    bnb_d = nc.dram_tensor("bnb", [128, 8], f32, kind="ExternalInput")
    qkb_d = nc.dram_tensor("qkb", [128, 4], f32, kind="ExternalInput")
    vb_d = nc.dram_tensor("vbr", [128, C], f32, kind="ExternalInput")
    dnb_d = nc.dram_tensor("dnb", [128, 2], f32, kind="ExternalInput")
    g2r_d = nc.dram_tensor("g2r", [128, C], f32, kind="ExternalInput")
    b2r_d = nc.dram_tensor("b2r", [128, C], f32, kind="ExternalInput")

    out_d = nc.dram_tensor("out", [TO, C], f32, kind="ExternalOutput")

    with tile.TileContext(nc) as tc, ExitStack() as octx:
        consts = octx.enter_context(tc.tile_pool(name="consts", bufs=1))
        persist = octx.enter_context(tc.tile_pool(name="persist", bufs=1))

        eps_sb = consts.tile([128, 1], f32)
        nc.vector.memset(eps_sb[:], 1e-5)
        ident = consts.tile([128, 128], bf)
        make_identity(nc, ident[:])

        wqk_sb = consts.tile([128, 2, 2 * C], bf)
        nc.sync.dma_start(wqk_sb[:], wqk_d[:, :].rearrange("(k p) o -> p k o", k=2))
        wv_sb = consts.tile([128, 2, C], bf)
        nc.sync.dma_start(wv_sb[:], wv_d[:, :].rearrange("(k p) o -> p k o", k=2))
        wproj_sb = consts.tile([128, 2, C], bf)
        nc.sync.dma_start(wproj_sb[:], wproj_d[:, :].rearrange("(k p) o -> p k o", k=2))
        expb_sb = consts.tile([128, 512], bf)
        nc.sync.dma_start(expb_sb[:], expb_d[:, :])
        wup_sb = consts.tile([128, 2, HID], bf)
        nc.sync.dma_start(wup_sb[:], wup_d[:, :].rearrange("(k p) o -> p k o", k=2))
        wdn_sb = consts.tile([128, 8, C], bf)
        nc.sync.dma_start(wdn_sb[:], wdn_d[:, :].rearrange("(k p) o -> p k o", k=8))
        dwdiag_sb = consts.tile([128, 9 * 8 * 32], bf)
        nc.sync.dma_start(dwdiag_sb[:], dwdiag_d[:, :])
        dwcol_sb = consts.tile([128, 8 * 9], f32)
        nc.sync.dma_start(dwcol_sb[:], dwcol_d[:, :])
        dwfull_sb = consts.tile([128, len(DW_POOL) * 9 * 512], bf)
        nc.sync.dma_start(dwfull_sb[:], dwfull_d[:, :])
        upb_sb = consts.tile([128, 8], f32)
        nc.sync.dma_start(upb_sb[:], upb_d[:, :])
        bnb_sb = consts.tile([128, 8], f32)
        nc.sync.dma_start(bnb_sb[:], bnb_d[:, :])
        qkb_sb = consts.tile([128, 4], f32)
        nc.sync.dma_start(qkb_sb[:], qkb_d[:, :])
        vb_sb = consts.tile([128, C], f32)
        nc.sync.dma_start(vb_sb[:], vb_d[:, :])
        dnb_sb = consts.tile([128, 2], f32)
        nc.sync.dma_start(dnb_sb[:], dnb_d[:, :])
        g2r_sb = consts.tile([128, C], f32)
        b2r_sb = consts.tile([128, C], f32)
        if sc2_affine:
            nc.sync.dma_start(g2r_sb[:], g2r_d[:, :])
            nc.sync.dma_start(b2r_sb[:], b2r_d[:, :])

        xn2T = [persist.tile([128, TK], bf, tag=f"xn2T{ct}", name=f"xn2T{ct}") for ct in range(2)]
        # row-major persistent store of LN2 output: [128 x-parts, KROWS*C]
        sc2_sb = persist.tile([128, KROWS * C], bf, tag="sc2sb", name="sc2sb")
        # per-pixel LN2 stats for xattn reconstruction: xattn = xn2*sigma + m
        # (sg_sb holds sigma+1 when not affine, sigma when affine)
        m_sb = persist.tile([128, AROWS], f32, tag="m_sb", name="m_sb")
        sg_sb = persist.tile([128, AROWS], f32, tag="sg_sb", name="sg_sb")

        # ======================= STAGE A =======================
        with ExitStack() as actx:
            xrow_p = actx.enter_context(tc.tile_pool(name="xrow", bufs=4))
            ln_p = actx.enter_context(tc.tile_pool(name="ln", bufs=4))
            mv_p = actx.enter_context(tc.tile_pool(name="mv", bufs=4))
            xnT_p = actx.enter_context(tc.tile_pool(name="xnT", bufs=2))
            qk_p = actx.enter_context(tc.tile_pool(name="qk", bufs=2))
            v_p = actx.enter_context(tc.tile_pool(name="vp", bufs=10))
            at_p = actx.enter_context(tc.tile_pool(name="at", bufs=2))
            small_p = actx.enter_context(tc.tile_pool(name="small", bufs=6))
            proj_p = actx.enter_context(tc.tile_pool(name="proj", bufs=2))
            oT_p = actx.enter_context(tc.tile_pool(name="oTp", bufs=9))
            xat_p = actx.enter_context(tc.tile_pool(name="xatp", bufs=9))
            tail_p = actx.enter_context(tc.tile_pool(name="tail", bufs=3))
            ps_tp = actx.enter_context(tc.tile_pool(name="ps_tp", bufs=2, space="PSUM"))
            ps_mm = actx.enter_context(tc.tile_pool(name="ps_mm", bufs=2, space="PSUM"))
            ps_sc = actx.enter_context(tc.tile_pool(name="ps_sc", bufs=2, space="PSUM"))
            ps_pat = actx.enter_context(tc.tile_pool(name="ps_pat", bufs=1, space="PSUM"))
            ps_oT = actx.enter_context(tc.tile_pool(name="ps_oT", bufs=1, space="PSUM"))

            for wr in range(NWR):
                xq = [xrow_p.tile([128, 4, C], f32, tag=f"xq{i}", name=f"xq{i}")
                      for i in range(2)]
                for h4 in range(2):
                    y0 = 8 * wr + 4 * h4
                    nc.sync.dma_start(
                        xq[h4][:],
                        xs_d[y0 * WI:(y0 + 4) * WI, :]
                        .rearrange("(r p) c -> p r c", p=WI))
                xrows = [xq[ry // 4][:, ry % 4, :] for ry in range(8)]
                xnT = [xnT_p.tile([128, 1024], bf, tag=f"xnT{ct}", name=f"xnT{ct}") for ct in range(2)]
                mv8 = mv_p.tile([128, 8, 2], f32, tag="mv8", name="mv8")
                rstd8 = mv_p.tile([128, 8], f32, tag="rstd8", name="rstd8")
                lvar8 = mv_p.tile([128, 8], f32, tag="lvar8", name="lvar8")
                for ry in range(8):
                    st = mv_p.tile([128, 6], f32, tag="st", name="st")
                    nc.vector.bn_stats(st[:], xrows[ry])
                    nc.vector.bn_aggr(mv8[:, ry, :], st[:])
                nc.scalar.activation(lvar8[:], mv8[:, :, 1], ACTF.Ln,
                                     bias=eps_sb[:], scale=1.0)
                nc.scalar.activation(rstd8[:], lvar8[:], ACTF.Exp, scale=-0.5)
                mrs8 = mv_p.tile([128, 8], f32, tag="mrs8", name="mrs8")
                nc.vector.tensor_tensor(out=mrs8[:], in0=mv8[:, :, 0],
                                        in1=rstd8[:], op=ALU.mult)
                for ry in range(8):
                    # xnb = (x - m) * rstd = x*rstd - m*rstd, fused
                    xnb = ln_p.tile([128, C], bf, tag="xnb", name="xnb")
                    nc.vector.scalar_tensor_tensor(
                        out=xnb[:], in0=xrows[ry], scalar=rstd8[:, ry:ry + 1],
                        in1=_sub_ap(mrs8[:], 0, 128, ry, [[0, C]]),
                        op0=ALU.mult, op1=ALU.subtract)
                    for ct in range(2):
                        tp = ps_tp.tile([128, 512], bf, tag="ps", name="ps")
                        nc.tensor.transpose(tp[:, 0:128],
                                            xnb[:, 128 * ct:128 * ct + 128], ident[:])
                        # scatter row-major pixel row -> window-ordered columns
                        nc.vector.tensor_copy(
                            _sub_ap(xnT[ct][:], 0, 128, 8 * ry, [[64, 16], [1, 8]]),
                            tp[:, 0:128])

                # ---- q/k projections: [oc-tile][128, 1024 tok(window-order)]
                qkT = [qk_p.tile([128, 1024], bf, tag=f"qkT{m}", name=f"qkT{m}") for m in range(4)]
                for m in range(4):
                    for j in range(2):
                        pqk = ps_mm.tile([128, 512], f32, tag="ps", name="ps")
                        for kt in range(2):
                            rhs = xnT[kt][:, 512 * j:512 * j + 512]
                            nc.tensor.matmul(pqk[:],
                                             lhsT=wqk_sb[:, kt, 128 * m:128 * m + 128],
                                             rhs=rhs, start=(kt == 0), stop=(kt == 1))
                        if qk_bias_nz:
                            nc.vector.tensor_scalar_add(
                                qkT[m][:, 512 * j:512 * j + 512], pqk[:],
                                qkb_sb[:, m:m + 1])
                        else:
                            nc.vector.tensor_copy(qkT[m][:, 512 * j:512 * j + 512],
                                                  pqk[:])

                # ---- v per window-pair: [64 tok, 512] (win-major, bf16)
                vsbs = []
                for u in range(8):
                    pv = ps_mm.tile([128, 512], f32, tag="ps", name="ps")
                    for w in range(2):
                        for kt in range(2):
                            lhsT = xnT[kt][:, 64 * (2 * u + w):64 * (2 * u + w) + 64]
                            nc.tensor.matmul(pv[0:64, 256 * w:256 * w + 256],
                                             lhsT=lhsT, rhs=wv_sb[:, kt, :],
                                             start=(kt == 0), stop=(kt == 1))
                    v2 = v_p.tile([128, 512], bf, tag="v2", name="v2")
                    if v_bias_nz:
                        nc.vector.tensor_tensor(
                            out=v2[0:64, :], in0=pv[0:64, :],
                            in1=_sub_ap(vb_sb[:], 0, 64, 0, [[0, 2], [1, C]]),
                            op=ALU.add)
                    else:
                        nc.scalar.copy(v2[0:64, :], pv[0:64, :])
                    vsbs.append(v2)

                # ---- attention per window-pair
                oTs = []
                for u in range(8):
                    pscs = [ps_sc.tile([128, 512], f32, tag="ps", name="ps") for _ in range(4)]
                    for h in range(NH):
                        g, jh = h % 4, h // 4
                        qt_t = qkT[h // 4]
                        kt_t = qkT[2 + h // 4]
                        for w in range(2):
                            tok0 = 64 * (2 * u + w)
                            nc.tensor.matmul(
                                pscs[g][64 * w:64 * w + 64, 64 * jh:64 * jh + 64],
                                lhsT=qt_t[32 * g:32 * g + 32, tok0:tok0 + 64],
                                rhs=kt_t[32 * g:32 * g + 32, tok0:tok0 + 64],
                                start=True, stop=True,
                                tile_position=(32 * g, 64 * w))
                    attn_e = at_p.tile([128, 512], bf, tag="attn_e", name="attn_e")
                    for g in range(4):
                        nc.scalar.activation(attn_e[:, 128 * g:128 * g + 128],
                                             pscs[g][:, 0:128], ACTF.Exp)
                    attn_u = at_p.tile([128, 512], bf, tag="attn_u", name="attn_u")
                    nc.gpsimd.tensor_tensor(out=attn_u[:], in0=attn_e[:],
                                            in1=expb_sb[:], op=ALU.mult)
                    r8 = small_p.tile([128, 8], f32, tag="r8", name="r8")
                    nc.vector.tensor_reduce(
                        r8[:], attn_u[:].rearrange("p (a k) -> p a k", a=8),
                        axis=AX.X, op=ALU.add)
                    rr8 = small_p.tile([128, 8], f32, tag="rr8", name="rr8")
                    nc.vector.reciprocal(rr8[:], r8[:])
                    attn_n = at_p.tile([128, 512], bf, tag="attn_n", name="attn_n")
                    nc.vector.tensor_tensor(
                        out=attn_n[:], in0=attn_u[:],
                        in1=_sub_ap(rr8[:], 0, 128, 0, [[1, 8], [0, 64]]),
                        op=ALU.mult)
                    aT = []
                    for g in range(4):
                        pat = ps_pat.tile([128, 512], bf, tag="ps", name="ps")
                        for jh in range(2):
                            nc.tensor.transpose(
                                pat[0:64, 128 * jh:128 * jh + 128],
                                attn_n[:, 128 * g + 64 * jh:128 * g + 64 * jh + 64],
                                ident[:])
                        t = at_p.tile([128, 256], bf, tag=f"aT{g}", name=f"aT{g}")
                        if g >= 2:
                            nc.scalar.copy(t[0:64, :], pat[0:64, 0:256])
                        else:
                            nc.vector.tensor_copy(t[0:64, :], pat[0:64, 0:256])
                        aT.append(t)
                    poT = ps_oT.tile([128, 256], f32, tag="ps", name="ps")
                    for h in range(NH):
                        g, jh = h % 4, h // 4
                        for w in range(2):
                            nc.tensor.matmul(
                                poT[32 * g:32 * g + 32,
                                    128 * jh + 64 * w:128 * jh + 64 * w + 64],
                                lhsT=vsbs[u][0:64,
                                             256 * w + 32 * h:256 * w + 32 * h + 32],
                                rhs=aT[g][0:64, 128 * jh + 64 * w:128 * jh + 64 * w + 64],
                                start=True, stop=True,
                                tile_position=(0, 32 * g))
                    ot = oT_p.tile([128, 256], bf, tag="oT", name="oT")
                    nc.scalar.copy(ot[:], poT[:])
                    oTs.append(ot)

                # ---- proj: projT [oc-tile][128, 1024] window-order
                projT = [proj_p.tile([128, 1024], bf, tag=f"projT{m}", name=f"projT{m}") for m in range(2)]
                for m in range(2):
                    for jc in range(2):
                        pp = ps_mm.tile([128, 512], f32, tag="ps", name="ps")
                        for uu in range(4):
                            u = 4 * jc + uu
                            for kt in range(2):
                                nc.tensor.matmul(
                                    pp[:, 128 * uu:128 * uu + 128],
                                    lhsT=wproj_sb[:, kt, 128 * m:128 * m + 128],
                                    rhs=oTs[u][:, 128 * kt:128 * kt + 128],
                                    start=(kt == 0), stop=(kt == 1))
                        for w in range(2):
                            # psum cols (uu, w fixed, iy, ix) -> row-major
                            src = _sub_ap(pp[:], 0, 128, 64 * w,
                                          [[128, 4], [8, 8], [1, 8]])
                            dst = _sub_ap(projT[m][:], 0, 128,
                                          8 * (8 * jc + w), [[16, 4], [128, 8], [1, 8]])
                            nc.vector.tensor_copy(dst, src)

                # ---- tail: per kept row
                if wr == 0:
                    keep = [7]
                elif wr == NWR - 1:
                    keep = [0]
                else:
                    keep = list(range(8))
                mvk = mv_p.tile([128, 8, 2], f32, tag="mvk", name="mvk")
                if wr in (0, NWR - 1):
                    nc.vector.memset(mvk[:], 0.0)
                lvk = mv_p.tile([128, 8], f32, tag="lvk", name="lvk")
                rsk = mv_p.tile([128, 8], f32, tag="rsk", name="rsk")
                xat_tiles = {}
                for ry in keep:
                    kk = 8 * wr + ry - 7
                    pfin = ps_tp.tile([128, 1024], bf, tag="ps", name="ps")
                    for m in range(2):
                        nc.tensor.transpose(pfin[:, 128 * m:128 * m + 128],
                                            projT[m][:, 128 * ry:128 * ry + 128],
                                            ident[:])
                    xat = xat_p.tile([128, C], f32, tag="xat", name="xat")
                    nc.vector.tensor_tensor(out=xat[:], in0=pfin[:, 0:256],
                                            in1=xrows[ry], op=ALU.add)
                    st2 = mv_p.tile([128, 6], f32, tag="st2", name="st2")
                    nc.vector.bn_stats(st2[:], xat[:])
                    nc.vector.bn_aggr(mvk[:, ry, :], st2[:])
                    xat_tiles[ry] = xat
                nc.scalar.activation(lvk[:], mvk[:, :, 1], ACTF.Ln,
                                     bias=eps_sb[:], scale=1.0)
                nc.scalar.activation(rsk[:], lvk[:], ACTF.Exp, scale=-0.5)
                k0, k1 = keep[0], keep[-1] + 1
                sig8 = mv_p.tile([128, 8], f32, tag="sig8", name="sig8")
                nc.scalar.activation(sig8[:, k0:k1], lvk[:, k0:k1], ACTF.Exp,
                                     scale=0.5)
                nc.vector.tensor_copy(
                    m_sb[:, 8 * wr + k0:8 * wr + k1],
                    mvk[:, k0:k1, 0])
                nc.vector.tensor_scalar_add(
                    sg_sb[:, 8 * wr + k0:8 * wr + k1], sig8[:, k0:k1],
                    0.0 if sc2_affine else 1.0)
                mrsk = mv_p.tile([128, 8], f32, tag="mrsk", name="mrsk")
                nc.vector.tensor_tensor(out=mrsk[:], in0=mvk[:, :, 0],
                                        in1=rsk[:], op=ALU.mult)
                for ry in keep:
                    kk = 8 * wr + ry - 7
                    xat = xat_tiles[ry]
                    # sc2_sb always holds the PRE-affine LN2 output (bf16)
                    xn2s = _sub_ap(sc2_sb[:], 0, 128, kk * C, [[1, C]])
                    nc.vector.scalar_tensor_tensor(
                        out=xn2s, in0=xat[:], scalar=rsk[:, ry:ry + 1],
                        in1=_sub_ap(mrsk[:], 0, 128, ry, [[0, C]]),
                        op0=ALU.mult, op1=ALU.subtract)
                    for ct in range(2):
                        tp2 = ps_tp.tile([128, 512], bf, tag="ps", name="ps2")
                        nc.tensor.transpose(
                            tp2[:, 0:128],
                            _sub_ap(sc2_sb[:], 0, 128, kk * C + 128 * ct, [[1, 128]]),
                            ident[:])
                        nc.vector.tensor_copy(xn2T[ct][:, 128 * kk:128 * kk + 128],
                                              tp2[:, 0:128])

        # ======================= STAGE B =======================
        UW = 130
        with ExitStack() as bctx:
            u_p = bctx.enter_context(tc.tile_pool(name="u_p", bufs=9))
            r_p = bctx.enter_context(tc.tile_pool(name="r_p", bufs=9))
            dnsb_p = bctx.enter_context(tc.tile_pool(name="dnsb", bufs=3))
            fin_p = bctx.enter_context(tc.tile_pool(name="fin", bufs=2))
            psU = bctx.enter_context(tc.tile_pool(name="psU", bufs=2, space="PSUM"))
            psR = bctx.enter_context(tc.tile_pool(name="psR", bufs=2, space="PSUM"))
            psD = bctx.enter_context(tc.tile_pool(name="psD", bufs=2, space="PSUM"))
            psF = bctx.enter_context(tc.tile_pool(name="psF", bufs=2, space="PSUM"))

            for q in range(4):
                kk0 = 16 * q
                Us = []
                for b in range(8):
                    U = u_p.tile([128, 18 * UW], bf, tag="U", name="U")
                    nc.vector.memset(
                        _sub_ap(U[:], 0, 128, 0, [[UW, 18], [129, 2]]), 0.0)
                    for i0 in range(0, 18, 4):
                        nrow = min(4, 18 - i0)
                        pu = psU.tile([128, 512], f32, tag="ps", name="ps")
                        for kt in range(2):
                            nc.tensor.matmul(
                                pu[:, 0:128 * nrow],
                                lhsT=wup_sb[:, kt, 128 * b:128 * b + 128],
                                rhs=xn2T[kt][:, (kk0 + i0) * WI:(kk0 + i0 + nrow) * WI],
                                start=(kt == 0), stop=(kt == 1))
                        nc.scalar.activation(
                            _sub_ap(U[:], 0, 128, i0 * UW + 1, [[UW, nrow], [1, 128]]),
                            pu[:, 0:128 * nrow], ACTF.Relu, bias=upb_sb[:, b:b + 1])
                    Us.append(U)
                for s4 in range(4):
                    jj0 = 4 * s4
                    Rs = []
                    for b in range(8):
                        R = r_p.tile([128, 512], bf, tag="R", name="R")
                        if b in DW_PE:
                            pr = psR.tile([128, 512], f32, tag="ps", name="ps")
                            for s in range(4):
                                for t in range(9):
                                    dy, dx = t // 3, t % 3
                                    lhsT = dwdiag_sb[
                                        32 * s:32 * s + 32,
                                        (t * 8 + b) * 32:(t * 8 + b) * 32 + 32]
                                    rhs = _sub_ap(Us[b][:], 32 * s, 32,
                                                  (jj0 + dy) * UW + dx,
                                                  [[UW, 4], [1, 128]])
                                    nc.tensor.matmul(
                                        pr[32 * s:32 * s + 32, 0:512],
                                        lhsT=lhsT, rhs=rhs,
                                        start=(t == 0), stop=(t == 8),
                                        tile_position=(32 * s, 32 * s))
                            nc.scalar.activation(R[:], pr[:], ACTF.Relu,
                                                 bias=bnb_sb[:, b:b + 1])
                        elif b in DW_DVE:
                            acc = [r_p.tile([128, 512], bf, tag=f"acc{i}", bufs=2,
                                            name=f"acc{i}") for i in range(2)]
                            for t in range(9):
                                dy, dx = t // 3, t % 3
                                u_in = _sub_ap(Us[b][:], 0, 128,
                                               (jj0 + dy) * UW + dx,
                                               [[UW, 4], [1, 128]])
                                wcol = dwcol_sb[:, 9 * b + t:9 * b + t + 1]
                                if t == 0:
                                    nc.vector.tensor_scalar_mul(acc[0][:], u_in, wcol)
                                else:
                                    nc.vector.scalar_tensor_tensor(
                                        out=acc[t % 2][:], in0=u_in,
                                        scalar=wcol, in1=acc[(t + 1) % 2][:],
                                        op0=ALU.mult, op1=ALU.add)
                            nc.scalar.activation(R[:], acc[0][:], ACTF.Relu,
                                                 bias=bnb_sb[:, b:b + 1])
                        else:
                            bp = DW_POOL.index(b)
                            acc = [r_p.tile([128, 512], bf, tag=f"pac{i}", bufs=2,
                                            name=f"pac{i}") for i in range(3)]
                            for t in range(9):
                                dy, dx = t // 3, t % 3
                                u_in = _sub_ap(Us[b][:], 0, 128,
                                               (jj0 + dy) * UW + dx,
                                               [[UW, 4], [1, 128]])
                                wfull = dwfull_sb[:, (9 * bp + t) * 512:
                                                  (9 * bp + t) * 512 + 512]
                                if t == 0:
                                    nc.gpsimd.tensor_tensor(
                                        out=acc[0][:], in0=u_in, in1=wfull,
                                        op=ALU.mult)
                                else:
                                    tmp = acc[2]
                                    nc.gpsimd.tensor_tensor(
                                        out=tmp[:], in0=u_in, in1=wfull,
                                        op=ALU.mult)
                                    nc.gpsimd.tensor_tensor(
                                        out=acc[t % 2][:], in0=tmp[:],
                                        in1=acc[(t + 1) % 2][:], op=ALU.add)
                            nc.scalar.activation(R[:], acc[0][:], ACTF.Relu,
                                                 bias=bnb_sb[:, b:b + 1])
                        Rs.append(R)
                    dn = dnsb_p.tile([128, 2, 512], bf, tag="dn", name="dn")
                    for m in range(2):
                        pd = psD.tile([128, 512], f32, tag="ps", name="ps")
                        for b in range(8):
                            nc.tensor.matmul(pd[:],
                                             lhsT=wdn_sb[:, b, 128 * m:128 * m + 128],
                                             rhs=Rs[b][:], start=(b == 0),
                                             stop=(b == 7))
                        if dn_bias_nz:
                            nc.vector.tensor_scalar_add(dn[:, m, :], pd[:],
                                                        dnb_sb[:, m:m + 1])
                        else:
                            nc.scalar.copy(dn[:, m, :], pd[:])
                    pf = psF.tile([128, 1024], bf, tag="pf", name="pf")
                    for jj in range(4):
                        for m in range(2):
                            nc.tensor.transpose(
                                pf[:, 256 * jj + 128 * m:256 * jj + 128 * m + 128],
                                dn[:, m, 128 * jj:128 * jj + 128], ident[:])
                    j0 = 16 * q + jj0
                    kkf = j0 + 1
                    y0 = kkf + 7   # global pixel row of first output row
                    sc4 = _sub_ap(sc2_sb[:], 0, 128, kkf * C, [[1, 4 * C]])
                    # a2 = xn2*(sigma+1) + m   (== sc + xattn when not affine)
                    a1 = fin_p.tile([128, 1024], f32, tag="a1", name="a1")
                    nc.gpsimd.tensor_tensor(
                        out=a1[:], in0=sc4,
                        in1=_sub_ap(sg_sb[:], 0, 128, y0, [[1, 4], [0, C]]),
                        op=ALU.mult)
                    a2 = fin_p.tile([128, 1024], f32, tag="a2", name="a2")
                    nc.gpsimd.tensor_tensor(
                        out=a2[:], in0=a1[:],
                        in1=_sub_ap(m_sb[:], 0, 128, y0, [[1, 4], [0, C]]),
                        op=ALU.add)
                    if sc2_affine:
                        # a2 = xattn only; shortcut = xn2*g2 + b2 added here
                        a3 = fin_p.tile([128, 1024], f32, tag="a3", name="a3")
                        nc.gpsimd.tensor_tensor(
                            out=a3[:], in0=sc4,
                            in1=_sub_ap(g2r_sb[:], 0, 128, 0, [[0, 4], [1, C]]),
                            op=ALU.mult)
                        a4 = fin_p.tile([128, 1024], f32, tag="a4", name="a4")
                        nc.gpsimd.tensor_tensor(
                            out=a4[:], in0=a3[:],
                            in1=_sub_ap(b2r_sb[:], 0, 128, 0, [[0, 4], [1, C]]),
                            op=ALU.add)
                        a5 = fin_p.tile([128, 1024], f32, tag="a5", name="a5")
                        nc.vector.tensor_tensor(out=a5[:], in0=a2[:], in1=a4[:],
                                                op=ALU.add)
                        a2 = a5
                    out4 = fin_p.tile([128, 1024], f32, tag="out4", name="out4")
                    nc.vector.tensor_tensor(
                        out=out4[:], in0=a2[:], in1=pf[:], op=ALU.add)
                    nc.sync.dma_start(
                        out_d[j0 * WI:(j0 + 4) * WI, :]
                        .rearrange("(r p) c -> p r c", p=WI),
                        out4[:].rearrange("p (r c) -> p r c", r=4))

    nc.compile()
    return nc


def _prep(g1, b1, qkv_w, qkv_b, rpb_table, rel_idx, proj_w, g2, b2,
          up_w, up_b, dw_w, bn_g, bn_b, down_w, down_b):
    f = np.float32
    g1 = np.asarray(g1, f); b1 = np.asarray(b1, f)
    qkv_w = np.asarray(qkv_w, f); qkv_b = np.asarray(qkv_b, f)
    rpb = np.asarray(rpb_table, f); ridx = np.asarray(rel_idx)
    proj_w = np.asarray(proj_w, f)
    g2 = np.asarray(g2, f); b2 = np.asarray(b2, f)
    up_w = np.asarray(up_w, f); up_b = np.asarray(up_b, f)
    dw_w = np.asarray(dw_w, f); bn_g = np.asarray(bn_g, f)
    bn_b = np.asarray(bn_b, f)
    down_w = np.asarray(down_w, f); down_b = np.asarray(down_b, f)

    sc = HD ** -0.5
    wq = qkv_w[:C] * g1[None, :] * sc
    wk = qkv_w[C:2 * C] * g1[None, :]
    wv = qkv_w[2 * C:] * g1[None, :]
    bq = (qkv_b[:C] + qkv_w[:C] @ b1) * sc
    bk = qkv_b[C:2 * C] + qkv_w[C:2 * C] @ b1
    bv = qkv_b[2 * C:] + qkv_w[2 * C:] @ b1

    wqk = np.concatenate([wq, wk], 0).T.astype(BF16).copy()
    wv_t = wv.T.astype(BF16).copy()
    wproj = proj_w.T.astype(BF16).copy()

    bias = rpb[np.asarray(ridx).reshape(-1)].reshape(64, 64, NH).transpose(2, 0, 1)
    expb = np.zeros((128, 512), f)
    for h in range(NH):
        cc = 128 * (h % 4) + 64 * (h // 4)
        eb = np.exp(bias[h])
        expb[0:64, cc:cc + 64] = eb
        expb[64:128, cc:cc + 64] = eb
    expb = expb.astype(BF16)

    wup = (up_w * g2[None, :]).T.astype(BF16).copy()
    upb = (up_b + up_w @ b2).astype(f)
    bns = bn_g * (1.0 + BN_EPS) ** -0.5
    dww = dw_w.reshape(HID, 9) * bns[:, None]
    dwdiag = np.zeros((128, 9 * 8 * 32), f)
    pp = np.arange(128)
    for b in range(8):
        for t in range(9):
            dwdiag[pp, (t * 8 + b) * 32 + (pp % 32)] = dww[128 * b + pp, t]
    dwdiag = dwdiag.astype(BF16)
    # per-partition tap weights for the DVE/Pool MAC path: dwcol[p, 9b+t]
    dwcol = np.zeros((128, 8 * 9), f)
    for b in range(8):
        dwcol[:, 9 * b:9 * b + 9] = dww[128 * b:128 * (b + 1), :]
    # free-dim-replicated tap weights for the Pool tensor_tensor path
    dwfull = np.zeros((128, len(DW_POOL) * 9 * 512), f)
    for bp, b in enumerate(DW_POOL):
        for t in range(9):
            dwfull[:, (9 * bp + t) * 512:(9 * bp + t + 1) * 512] = \
                dww[128 * b:128 * (b + 1), t:t + 1]
    dwfull = dwfull.astype(BF16)
    wdn = down_w.T.astype(BF16).copy()

    def col_n(v, n):
        return np.asarray(v, f).reshape(n, 128).T.copy()

    qkb = col_n(np.concatenate([bq, bk]), 4)
    vbr = np.broadcast_to(bv[None, :], (128, C)).astype(f).copy()
    dnb = col_n(down_b, 2)
    g2r = np.broadcast_to(g2[None, :], (128, C)).astype(f).copy()
    b2r = np.broadcast_to(b2[None, :], (128, C)).astype(f).copy()

    flags = (bool(np.any(qkb)), bool(np.any(bv)), bool(np.any(down_b)),
             not (np.allclose(g2, 1.0) and np.allclose(b2, 0.0)))

    consts = dict(wqk=wqk, wv=wv_t, wproj=wproj, expb=expb, wup=wup, wdn=wdn,
                  dwdiag=dwdiag, dwcol=dwcol, dwfull=dwfull,
                  upb=col_n(upb, 8),
                  bnb=col_n(bn_b, 8), qkb=qkb,
                  vbr=vbr, dnb=dnb, g2r=g2r, b2r=b2r)
    return consts, flags


def kernel(x, H, W, g1, b1, qkv_w, qkv_b, rpb_table, rel_idx, proj_w,
           g2, b2, up_w, up_b, dw_w, bn_g, bn_b, down_w, down_b):
    global LAST_RESULTS
    from concourse.bass_utils import run_bass_kernel_spmd

    x = np.asarray(x, np.float32)
    consts, flags = _prep(g1, b1, qkv_w, qkv_b, rpb_table, rel_idx, proj_w,
                          g2, b2, up_w, up_b, dw_w, bn_g, bn_b, down_w, down_b)
    if flags not in _BUILD_CACHE:
        _BUILD_CACHE[flags] = _build(flags)
    nc = _BUILD_CACHE[flags]

    ximg = x.reshape(B_, HI, WI, C)
    in_maps = []
    for core in range(NCORES):
        b, top = core // 2, (core % 2 == 0)
        r0 = 0 if top else 64
        xs = np.zeros((AROWS, WI, C), np.float32)
        lo, hi = r0 - 8, r0 + 72
        slo, shi = max(lo, 0), min(hi, HI)
        xs[slo - lo:shi - lo] = ximg[b, slo:shi]
        m = {"xs": xs.reshape(TA, C)}
        m.update(consts)
        in_maps.append(m)

    res = run_bass_kernel_spmd(nc, in_maps, core_ids=list(range(NCORES)))
    LAST_RESULTS = res

    out = np.empty((B_, HI, WI, C), np.float32)
    for core in range(NCORES):
        b, top = core // 2, (core % 2 == 0)
        r0 = 0 if top else 64
        out[b, r0:r0 + 64] = res.results[core]["out"].reshape(OROWS, WI, C)
    return out.reshape(B_, HI * WI, C)


# revision 8
# speedup vs baseline: 1.4311x; 1.3988x over previous
"""Swin-style basic block (W-MSA + CNN-MLP) Trainium2 kernel, 8-way sharded.

vs baseline: LN2 output kept in SBUF (bf16) with the attention residual
reconstructed from per-pixel LN stats (no DRAM round trips); depthwise 3x3
split across TensorE (N=512 diagonal matmuls) / GpSimd (tensor_tensor MAC) /
VectorE (fused scalar_tensor_tensor MAC); bn+relu fused into ScalarE
activations; fused LN normalize; single-op softmax normalize; LN rstd via
DVE-reciprocal + ScalarE Sqrt (one activation-table set, no Ln/Exp table
thrash); double-banked attention transposes; many PSUM evacuations moved to
the otherwise-idle ScalarE; quad-row input DMA.

Sharding: 8 shards = (batch b in 0..3) x (top/bottom half of the 128x128
image).  Each core receives 10 window-rows of input (80 pixel rows: its own
64 plus one full window-row of halo above and below, zero-padded outside the
image).
"""

import numpy as np
import ml_dtypes
from contextlib import ExitStack

B_, HI, WI, C = 4, 128, 128, 256
WS, NH, HD = 8, 8, 32
HID = 1024
BN_EPS = 1e-5
NCORES = 8
NWR = 10            # window-rows per core (8 own + 2 halo)
AROWS = 8 * NWR     # 80
KROWS = 66          # kept x_attn rows: local pixel rows 7..73
OROWS = 64
TA = AROWS * WI     # 10240
TK = KROWS * WI     # 8448
TO = OROWS * WI     # 8192

BF16 = ml_dtypes.bfloat16

# depthwise-conv engine split by 128-channel subgroup b (0..7)
DW_PE = (0, 1)               # diagonal matmuls on TensorE
DW_POOL = (2, 3)             # mult+add tensor_tensor chain on GpSimd
DW_DVE = (4, 5, 6, 7)        # fused MAC (scalar_tensor_tensor) on VectorE

_BUILD_CACHE = {}
LAST_RESULTS = None


def _sub_ap(base, part0, nparts, free_off, free_dims):
    import concourse.bass as bass
    pstride = base.ap[0][0]
    return bass.AP(
        tensor=base.tensor,
        offset=base.offset + part0 * pstride + free_off,
        ap=[[pstride, nparts]] + [list(d) for d in free_dims],
    )


def _build(flags):
    import concourse.bass as bass
    import concourse.tile as tile
    from concourse import bacc, mybir
    from concourse.masks import make_identity

    qk_bias_nz, v_bias_nz, dn_bias_nz, sc2_affine = flags
    f32 = mybir.dt.float32
    bf = mybir.dt.bfloat16
    ALU = mybir.AluOpType
    ACTF = mybir.ActivationFunctionType
    AX = mybir.AxisListType

    nc = bacc.Bacc("TRN2", target_bir_lowering=False, debug=False,
                   num_devices=NCORES)

    # ---------------- DRAM tensors ----------------
    xs_d = nc.dram_tensor("xs", [TA, C], f32, kind="ExternalInput")
    wqk_d = nc.dram_tensor("wqk", [C, 2 * C], bf, kind="ExternalInput")
    wv_d = nc.dram_tensor("wv", [C, C], bf, kind="ExternalInput")
    wproj_d = nc.dram_tensor("wproj", [C, C], bf, kind="ExternalInput")
    expb_d = nc.dram_tensor("expb", [128, 512], bf, kind="ExternalInput")
    wup_d = nc.dram_tensor("wup", [C, HID], bf, kind="ExternalInput")
    wdn_d = nc.dram_tensor("wdn", [HID, C], bf, kind="ExternalInput")
    dwdiag_d = nc.dram_tensor("dwdiag", [128, 9 * 8 * 32], bf, kind="ExternalInput")
    dwcol_d = nc.dram_tensor("dwcol", [128, 8 * 9], f32, kind="ExternalInput")
    dwfull_d = nc.dram_tensor("dwfull", [128, len(DW_POOL) * 9 * 512], bf,
                              kind="ExternalInput")
    upb_d = nc.dram_tensor("upb", [128, 8], f32, kind="ExternalInput")
    bnb_d = nc.dram_tensor("bnb", [128, 8], f32, kind="ExternalInput")
    qkb_d = nc.dram_tensor("qkb", [128, 4], f32, kind="ExternalInput")
    vb_d = nc.dram_tensor("vbr", [128, C], f32, kind="ExternalInput")
    dnb_d = nc.dram_tensor("dnb", [128, 2], f32, kind="ExternalInput")
    g2r_d = nc.dram_tensor("g2r", [128, C], f32, kind="ExternalInput")
    b2r_d = nc.dram_tensor("b2r", [128, C], f32, kind="ExternalInput")

    out_d = nc.dram_tensor("out", [TO, C], f32, kind="ExternalOutput")

    with tile.TileContext(nc) as tc, ExitStack() as octx:
        consts = octx.enter_context(tc.tile_pool(name="consts", bufs=1))
        persist = octx.enter_context(tc.tile_pool(name="persist", bufs=1))

        eps_sb = consts.tile([128, 1], f32)
        nc.vector.memset(eps_sb[:], 1e-5)
        ident = consts.tile([128, 128], bf)
        make_identity(nc, ident[:])

        wqk_sb = consts.tile([128, 2, 2 * C], bf)
        nc.sync.dma_start(wqk_sb[:], wqk_d[:, :].rearrange("(k p) o -> p k o", k=2))
        wv_sb = consts.tile([128, 2, C], bf)
        nc.sync.dma_start(wv_sb[:], wv_d[:, :].rearrange("(k p) o -> p k o", k=2))
        wproj_sb = consts.tile([128, 2, C], bf)
        nc.sync.dma_start(wproj_sb[:], wproj_d[:, :].rearrange("(k p) o -> p k o", k=2))
        expb_sb = consts.tile([128, 512], bf)
        nc.sync.dma_start(expb_sb[:], expb_d[:, :])
        wup_sb = consts.tile([128, 2, HID], bf)
        nc.sync.dma_start(wup_sb[:], wup_d[:, :].rearrange("(k p) o -> p k o", k=2))
        wdn_sb = consts.tile([128, 8, C], bf)
        nc.sync.dma_start(wdn_sb[:], wdn_d[:, :].rearrange("(k p) o -> p k o", k=8))
        dwdiag_sb = consts.tile([128, 9 * 8 * 32], bf)
        nc.sync.dma_start(dwdiag_sb[:], dwdiag_d[:, :])
        dwcol_sb = consts.tile([128, 8 * 9], f32)
        nc.sync.dma_start(dwcol_sb[:], dwcol_d[:, :])
        dwfull_sb = consts.tile([128, len(DW_POOL) * 9 * 512], bf)
        nc.sync.dma_start(dwfull_sb[:], dwfull_d[:, :])
        upb_sb = consts.tile([128, 8], f32)
        nc.sync.dma_start(upb_sb[:], upb_d[:, :])
        bnb_sb = consts.tile([128, 8], f32)
        nc.sync.dma_start(bnb_sb[:], bnb_d[:, :])
        qkb_sb = consts.tile([128, 4], f32)
        nc.sync.dma_start(qkb_sb[:], qkb_d[:, :])
        vb_sb = consts.tile([128, C], f32)
        nc.sync.dma_start(vb_sb[:], vb_d[:, :])
        dnb_sb = consts.tile([128, 2], f32)
        nc.sync.dma_start(dnb_sb[:], dnb_d[:, :])
        g2r_sb = consts.tile([128, C], f32)
        b2r_sb = consts.tile([128, C], f32)
        if sc2_affine:
            nc.sync.dma_start(g2r_sb[:], g2r_d[:, :])
            nc.sync.dma_start(b2r_sb[:], b2r_d[:, :])

        xn2T = [persist.tile([128, TK], bf, tag=f"xn2T{ct}", name=f"xn2T{ct}") for ct in range(2)]
        # row-major persistent store of LN2 output: [128 x-parts, KROWS*C]
        sc2_sb = persist.tile([128, KROWS * C], bf, tag="sc2sb", name="sc2sb")
        # per-pixel LN2 stats for xattn reconstruction: xattn = xn2*sigma + m
        # (sg_sb holds sigma+1 when not affine, sigma when affine)
        m_sb = persist.tile([128, AROWS], f32, tag="m_sb", name="m_sb")
        sg_sb = persist.tile([128, AROWS], f32, tag="sg_sb", name="sg_sb")

        # ======================= STAGE A =======================
        with ExitStack() as actx:
            xrow_p = actx.enter_context(tc.tile_pool(name="xrow", bufs=3))
            ln_p = actx.enter_context(tc.tile_pool(name="ln", bufs=4))
            mv_p = actx.enter_context(tc.tile_pool(name="mv", bufs=4))
            xnT_p = actx.enter_context(tc.tile_pool(name="xnT", bufs=2))
            qk_p = actx.enter_context(tc.tile_pool(name="qk", bufs=2))
            v_p = actx.enter_context(tc.tile_pool(name="vp", bufs=10))
            at_p = actx.enter_context(tc.tile_pool(name="at", bufs=3))
            small_p = actx.enter_context(tc.tile_pool(name="small", bufs=6))
            proj_p = actx.enter_context(tc.tile_pool(name="proj", bufs=2))
            oT_p = actx.enter_context(tc.tile_pool(name="oTp", bufs=9))
            xat_p = actx.enter_context(tc.tile_pool(name="xatp", bufs=9))
            tail_p = actx.enter_context(tc.tile_pool(name="tail", bufs=3))
            ps_tp = actx.enter_context(tc.tile_pool(name="ps_tp", bufs=2, space="PSUM"))
            ps_mm = actx.enter_context(tc.tile_pool(name="ps_mm", bufs=2, space="PSUM"))
            ps_sc = actx.enter_context(tc.tile_pool(name="ps_sc", bufs=2, space="PSUM"))
            ps_pat = actx.enter_context(tc.tile_pool(name="ps_pat", bufs=1, space="PSUM"))

            for wr in range(NWR):
                xq = [xrow_p.tile([128, 4, C], f32, tag=f"xq{i}", name=f"xq{i}")
                      for i in range(2)]
                for h4 in range(2):
                    y0 = 8 * wr + 4 * h4
                    nc.sync.dma_start(
                        xq[h4][:],
                        xs_d[y0 * WI:(y0 + 4) * WI, :]
                        .rearrange("(r p) c -> p r c", p=WI))
                xrows = [xq[ry // 4][:, ry % 4, :] for ry in range(8)]
                xnT = [xnT_p.tile([128, 1024], bf, tag=f"xnT{ct}", name=f"xnT{ct}") for ct in range(2)]
                mv8 = mv_p.tile([128, 8, 2], f32, tag="mv8", name="mv8")
                rstd8 = mv_p.tile([128, 8], f32, tag="rstd8", name="rstd8")
                lvar8 = mv_p.tile([128, 8], f32, tag="lvar8", name="lvar8")
                for ry in range(8):
                    st = mv_p.tile([128, 6], f32, tag="st", name="st")
                    nc.vector.bn_stats(st[:], xrows[ry])
                    nc.vector.bn_aggr(mv8[:, ry, :], st[:])
                vr8 = mv_p.tile([128, 8], f32, tag="vr8", name="vr8")
                nc.vector.tensor_scalar_add(vr8[:], mv8[:, :, 1], 1e-5)
                ir8 = mv_p.tile([128, 8], f32, tag="ir8", name="ir8")
                nc.vector.reciprocal(ir8[:], vr8[:])
                nc.scalar.activation(rstd8[:], ir8[:], ACTF.Sqrt)
                mrs8 = mv_p.tile([128, 8], f32, tag="mrs8", name="mrs8")
                nc.vector.tensor_tensor(out=mrs8[:], in0=mv8[:, :, 0],
                                        in1=rstd8[:], op=ALU.mult)
                for ry in range(8):
                    # xnb = (x - m) * rstd = x*rstd - m*rstd, fused
                    xnb = ln_p.tile([128, C], bf, tag="xnb", name="xnb")
                    nc.vector.scalar_tensor_tensor(
                        out=xnb[:], in0=xrows[ry], scalar=rstd8[:, ry:ry + 1],
                        in1=_sub_ap(mrs8[:], 0, 128, ry, [[0, C]]),
                        op0=ALU.mult, op1=ALU.subtract)
                    for ct in range(2):
                        tp = ps_tp.tile([128, 512], bf, tag="ps", name="ps")
                        nc.tensor.transpose(tp[:, 0:128],
                                            xnb[:, 128 * ct:128 * ct + 128], ident[:])
                        # scatter row-major pixel row -> window-ordered columns
                        nc.scalar.copy(
                            _sub_ap(xnT[ct][:], 0, 128, 8 * ry, [[64, 16], [1, 8]]),
                            tp[:, 0:128])

                # ---- q/k projections: [oc-tile][128, 1024 tok(window-order)]
                qkT = [qk_p.tile([128, 1024], bf, tag=f"qkT{m}", name=f"qkT{m}") for m in range(4)]
                for m in range(4):
                    for j in range(2):
                        pqk = ps_mm.tile([128, 512], f32, tag="ps", name="ps")
                        for kt in range(2):
                            rhs = xnT[kt][:, 512 * j:512 * j + 512]
                            nc.tensor.matmul(pqk[:],
                                             lhsT=wqk_sb[:, kt, 128 * m:128 * m + 128],
                                             rhs=rhs, start=(kt == 0), stop=(kt == 1))
                        if qk_bias_nz:
                            nc.vector.tensor_scalar_add(
                                qkT[m][:, 512 * j:512 * j + 512], pqk[:],
                                qkb_sb[:, m:m + 1])
                        else:
                            nc.scalar.copy(qkT[m][:, 512 * j:512 * j + 512],
                                           pqk[:])

                # ---- v per window-pair: [64 tok, 512] (win-major, bf16)
                vsbs = []
                for u in range(8):
                    pv = ps_mm.tile([128, 512], f32, tag="ps", name="ps")
                    for w in range(2):
                        for kt in range(2):
                            lhsT = xnT[kt][:, 64 * (2 * u + w):64 * (2 * u + w) + 64]
                            nc.tensor.matmul(pv[0:64, 256 * w:256 * w + 256],
                                             lhsT=lhsT, rhs=wv_sb[:, kt, :],
                                             start=(kt == 0), stop=(kt == 1))
                    v2 = v_p.tile([128, 512], bf, tag="v2", name="v2")
                    if v_bias_nz:
                        nc.vector.tensor_tensor(
                            out=v2[0:64, :], in0=pv[0:64, :],
                            in1=_sub_ap(vb_sb[:], 0, 64, 0, [[0, 2], [1, C]]),
                            op=ALU.add)
                    else:
                        nc.scalar.copy(v2[0:64, :], pv[0:64, :])
                    vsbs.append(v2)

                # ---- attention per window-pair
                oTs = []
                for u in range(8):
                    pscs = [ps_sc.tile([128, 512], f32, tag="ps", name="ps") for _ in range(4)]
                    for h in range(NH):
                        g, jh = h % 4, h // 4
                        qt_t = qkT[h // 4]
                        kt_t = qkT[2 + h // 4]
                        for w in range(2):
                            tok0 = 64 * (2 * u + w)
                            nc.tensor.matmul(
                                pscs[g][64 * w:64 * w + 64, 64 * jh:64 * jh + 64],
                                lhsT=qt_t[32 * g:32 * g + 32, tok0:tok0 + 64],
                                rhs=kt_t[32 * g:32 * g + 32, tok0:tok0 + 64],
                                start=True, stop=True,
                                tile_position=(32 * g, 64 * w))
                    attn_e = at_p.tile([128, 512], bf, tag="attn_e", name="attn_e")
                    for g in range(4):
                        nc.scalar.activation(attn_e[:, 128 * g:128 * g + 128],
                                             pscs[g][:, 0:128], ACTF.Exp)
                    attn_u = at_p.tile([128, 512], bf, tag="attn_u", name="attn_u")
                    nc.gpsimd.tensor_tensor(out=attn_u[:], in0=attn_e[:],
                                            in1=expb_sb[:], op=ALU.mult)
                    r8 = small_p.tile([128, 8], f32, tag="r8", name="r8")
                    nc.vector.tensor_reduce(
                        r8[:], attn_u[:].rearrange("p (a k) -> p a k", a=8),
                        axis=AX.X, op=ALU.add)
                    rr8 = small_p.tile([128, 8], f32, tag="rr8", name="rr8")
                    nc.vector.reciprocal(rr8[:], r8[:])
                    attn_n = at_p.tile([128, 512], bf, tag="attn_n", name="attn_n")
                    nc.vector.tensor_tensor(
                        out=attn_n[:], in0=attn_u[:],
                        in1=_sub_ap(rr8[:], 0, 128, 0, [[1, 8], [0, 64]]),
                        op=ALU.mult)
                    aT = []
                    for g in range(4):
                        pat = ps_pat.tile([128, 512], bf, tag=f"ps{g % 2}",
                                          bufs=1, name="ps")
                        for jh in range(2):
                            nc.tensor.transpose(
                                pat[0:64, 128 * jh:128 * jh + 128],
                                attn_n[:, 128 * g + 64 * jh:128 * g + 64 * jh + 64],
                                ident[:])
                        t = at_p.tile([128, 256], bf, tag=f"aT{g}", name=f"aT{g}")
                        if g >= 2:
                            nc.scalar.copy(t[0:64, :], pat[0:64, 0:256])
                        else:
                            nc.vector.tensor_copy(t[0:64, :], pat[0:64, 0:256])
                        aT.append(t)
                    poT = ps_pat.tile([128, 256], f32, tag="ps1", bufs=1, name="poT")
                    for h in range(NH):
                        g, jh = h % 4, h // 4
                        for w in range(2):
                            nc.tensor.matmul(
                                poT[32 * g:32 * g + 32,
                                    128 * jh + 64 * w:128 * jh + 64 * w + 64],
                                lhsT=vsbs[u][0:64,
                                             256 * w + 32 * h:256 * w + 32 * h + 32],
                                rhs=aT[g][0:64, 128 * jh + 64 * w:128 * jh + 64 * w + 64],
                                start=True, stop=True,
                                tile_position=(0, 32 * g))
                    ot = oT_p.tile([128, 256], bf, tag="oT", name="oT")
                    nc.scalar.copy(ot[:], poT[:])
                    oTs.append(ot)

                # ---- proj: projT [oc-tile][128, 1024] window-order
                projT = [proj_p.tile([128, 1024], bf, tag=f"projT{m}", name=f"projT{m}") for m in range(2)]
                for m in range(2):
                    for jc in range(2):
                        pp = ps_mm.tile([128, 512], f32, tag="ps", name="ps")
                        for uu in range(4):
                            u = 4 * jc + uu
                            for kt in range(2):
                                nc.tensor.matmul(
                                    pp[:, 128 * uu:128 * uu + 128],
                                    lhsT=wproj_sb[:, kt, 128 * m:128 * m + 128],
                                    rhs=oTs[u][:, 128 * kt:128 * kt + 128],
                                    start=(kt == 0), stop=(kt == 1))
                        for w in range(2):
                            # psum cols (uu, w fixed, iy, ix) -> row-major
                            src = _sub_ap(pp[:], 0, 128, 64 * w,
                                          [[128, 4], [8, 8], [1, 8]])
                            dst = _sub_ap(projT[m][:], 0, 128,
                                          8 * (8 * jc + w), [[16, 4], [128, 8], [1, 8]])
                            nc.scalar.copy(dst, src)

                # ---- tail: per kept row
                if wr == 0:
                    keep = [7]
                elif wr == NWR - 1:
                    keep = [0]
                else:
                    keep = list(range(8))
                mvk = mv_p.tile([128, 8, 2], f32, tag="mvk", name="mvk")
                if wr in (0, NWR - 1):
                    nc.vector.memset(mvk[:], 0.0)
                lvk = mv_p.tile([128, 8], f32, tag="lvk", name="lvk")
                rsk = mv_p.tile([128, 8], f32, tag="rsk", name="rsk")
                xat_tiles = {}
                for ry in keep:
                    kk = 8 * wr + ry - 7
                    pfin = ps_tp.tile([128, 1024], bf, tag="ps", name="ps")
                    for m in range(2):
                        nc.tensor.transpose(pfin[:, 128 * m:128 * m + 128],
                                            projT[m][:, 128 * ry:128 * ry + 128],
                                            ident[:])
                    xat = xat_p.tile([128, C], f32, tag="xat", name="xat")
                    nc.vector.tensor_tensor(out=xat[:], in0=pfin[:, 0:256],
                                            in1=xrows[ry], op=ALU.add)
                    st2 = mv_p.tile([128, 6], f32, tag="st2", name="st2")
                    nc.vector.bn_stats(st2[:], xat[:])
                    nc.vector.bn_aggr(mvk[:, ry, :], st2[:])
                    xat_tiles[ry] = xat
                k0, k1 = keep[0], keep[-1] + 1
                vpe = mv_p.tile([128, 8], f32, tag="vpe", name="vpe")
                nc.vector.tensor_scalar_add(vpe[:], mvk[:, :, 1], 1e-5)
                irk = mv_p.tile([128, 8], f32, tag="irk", name="irk")
                nc.vector.reciprocal(irk[:], vpe[:])
                nc.scalar.activation(rsk[:], irk[:], ACTF.Sqrt)
                sig8 = mv_p.tile([128, 8], f32, tag="sig8", name="sig8")
                nc.vector.tensor_tensor(out=sig8[:, k0:k1], in0=vpe[:, k0:k1],
                                        in1=rsk[:, k0:k1], op=ALU.mult)
                nc.vector.tensor_copy(
                    m_sb[:, 8 * wr + k0:8 * wr + k1],
                    mvk[:, k0:k1, 0])
                nc.vector.tensor_scalar_add(
                    sg_sb[:, 8 * wr + k0:8 * wr + k1], sig8[:, k0:k1],
                    0.0 if sc2_affine else 1.0)
                mrsk = mv_p.tile([128, 8], f32, tag="mrsk", name="mrsk")
                nc.vector.tensor_tensor(out=mrsk[:], in0=mvk[:, :, 0],
                                        in1=rsk[:], op=ALU.mult)
                for ry in keep:
                    kk = 8 * wr + ry - 7
                    xat = xat_tiles[ry]
                    # sc2_sb always holds the PRE-affine LN2 output (bf16)
                    xn2s = _sub_ap(sc2_sb[:], 0, 128, kk * C, [[1, C]])
                    nc.vector.scalar_tensor_tensor(
                        out=xn2s, in0=xat[:], scalar=rsk[:, ry:ry + 1],
                        in1=_sub_ap(mrsk[:], 0, 128, ry, [[0, C]]),
                        op0=ALU.mult, op1=ALU.subtract)
                    for ct in range(2):
                        tp2 = ps_tp.tile([128, 512], bf, tag="ps", name="ps2")
                        nc.tensor.transpose(
                            tp2[:, 0:128],
                            _sub_ap(sc2_sb[:], 0, 128, kk * C + 128 * ct, [[1, 128]]),
                            ident[:])
                        nc.scalar.copy(xn2T[ct][:, 128 * kk:128 * kk + 128],
                                       tp2[:, 0:128])

        # ======================= STAGE B =======================
        UW = 130
        with ExitStack() as bctx:
            u_p = bctx.enter_context(tc.tile_pool(name="u_p", bufs=9))
            r_p = bctx.enter_context(tc.tile_pool(name="r_p", bufs=9))
            dnsb_p = bctx.enter_context(tc.tile_pool(name="dnsb", bufs=3))
            fin_p = bctx.enter_context(tc.tile_pool(name="fin", bufs=2))
            psU = bctx.enter_context(tc.tile_pool(name="psU", bufs=2, space="PSUM"))
            psR = bctx.enter_context(tc.tile_pool(name="psR", bufs=2, space="PSUM"))
            psD = bctx.enter_context(tc.tile_pool(name="psD", bufs=2, space="PSUM"))
            psF = bctx.enter_context(tc.tile_pool(name="psF", bufs=2, space="PSUM"))

            for q in range(4):
                kk0 = 16 * q
                Us = []
                for b in range(8):
                    U = u_p.tile([128, 18 * UW], bf, tag="U", name="U")
                    nc.vector.memset(
                        _sub_ap(U[:], 0, 128, 0, [[UW, 18], [129, 2]]), 0.0)
                    for i0 in range(0, 18, 4):
                        nrow = min(4, 18 - i0)
                        pu = psU.tile([128, 512], f32, tag="ps", name="ps")
                        for kt in range(2):
                            nc.tensor.matmul(
                                pu[:, 0:128 * nrow],
                                lhsT=wup_sb[:, kt, 128 * b:128 * b + 128],
                                rhs=xn2T[kt][:, (kk0 + i0) * WI:(kk0 + i0 + nrow) * WI],
                                start=(kt == 0), stop=(kt == 1))
                        nc.scalar.activation(
                            _sub_ap(U[:], 0, 128, i0 * UW + 1, [[UW, nrow], [1, 128]]),
                            pu[:, 0:128 * nrow], ACTF.Relu, bias=upb_sb[:, b:b + 1])
                    Us.append(U)
                for s4 in range(4):
                    jj0 = 4 * s4
                    Rs = []
                    for b in range(8):
                        R = r_p.tile([128, 512], bf, tag="R", name="R")
                        if b in DW_PE:
                            pr = psR.tile([128, 512], f32, tag="ps", name="ps")
                            for s in range(4):
                                for t in range(9):
                                    dy, dx = t // 3, t % 3
                                    lhsT = dwdiag_sb[
                                        32 * s:32 * s + 32,
                                        (t * 8 + b) * 32:(t * 8 + b) * 32 + 32]
                                    rhs = _sub_ap(Us[b][:], 32 * s, 32,
                                                  (jj0 + dy) * UW + dx,
                                                  [[UW, 4], [1, 128]])
                                    nc.tensor.matmul(
                                        pr[32 * s:32 * s + 32, 0:512],
                                        lhsT=lhsT, rhs=rhs,
                                        start=(t == 0), stop=(t == 8),
                                        tile_position=(32 * s, 32 * s))
                            nc.scalar.activation(R[:], pr[:], ACTF.Relu,
                                                 bias=bnb_sb[:, b:b + 1])
                        elif b in DW_DVE:
                            acc = [r_p.tile([128, 512], bf, tag=f"acc{i}", bufs=2,
                                            name=f"acc{i}") for i in range(2)]
                            for t in range(9):
                                dy, dx = t // 3, t % 3
                                u_in = _sub_ap(Us[b][:], 0, 128,
                                               (jj0 + dy) * UW + dx,
                                               [[UW, 4], [1, 128]])
                                wcol = dwcol_sb[:, 9 * b + t:9 * b + t + 1]
                                if t == 0:
                                    nc.vector.tensor_scalar_mul(acc[0][:], u_in, wcol)
                                else:
                                    nc.vector.scalar_tensor_tensor(
                                        out=acc[t % 2][:], in0=u_in,
                                        scalar=wcol, in1=acc[(t + 1) % 2][:],
                                        op0=ALU.mult, op1=ALU.add)
                            nc.scalar.activation(R[:], acc[0][:], ACTF.Relu,
                                                 bias=bnb_sb[:, b:b + 1])
                        else:
                            bp = DW_POOL.index(b)
                            acc = [r_p.tile([128, 512], bf, tag=f"pac{i}", bufs=2,
                                            name=f"pac{i}") for i in range(3)]
                            for t in range(9):
                                dy, dx = t // 3, t % 3
                                u_in = _sub_ap(Us[b][:], 0, 128,
                                               (jj0 + dy) * UW + dx,
                                               [[UW, 4], [1, 128]])
                                wfull = dwfull_sb[:, (9 * bp + t) * 512:
                                                  (9 * bp + t) * 512 + 512]
                                if t == 0:
                                    nc.gpsimd.tensor_tensor(
                                        out=acc[0][:], in0=u_in, in1=wfull,
                                        op=ALU.mult)
                                else:
                                    tmp = acc[2]
                                    nc.gpsimd.tensor_tensor(
                                        out=tmp[:], in0=u_in, in1=wfull,
                                        op=ALU.mult)
                                    nc.gpsimd.tensor_tensor(
                                        out=acc[t % 2][:], in0=tmp[:],
                                        in1=acc[(t + 1) % 2][:], op=ALU.add)
                            nc.scalar.activation(R[:], acc[0][:], ACTF.Relu,
                                                 bias=bnb_sb[:, b:b + 1])
                        Rs.append(R)
                    dn = dnsb_p.tile([128, 2, 512], bf, tag="dn", name="dn")
                    for m in range(2):
                        pd = psD.tile([128, 512], f32, tag="ps", name="ps")
                        for b in range(8):
                            nc.tensor.matmul(pd[:],
                                             lhsT=wdn_sb[:, b, 128 * m:128 * m + 128],
                                             rhs=Rs[b][:], start=(b == 0),
                                             stop=(b == 7))
                        if dn_bias_nz:
                            nc.vector.tensor_scalar_add(dn[:, m, :], pd[:],
                                                        dnb_sb[:, m:m + 1])
                        else:
                            nc.scalar.copy(dn[:, m, :], pd[:])
                    pf = psF.tile([128, 1024], bf, tag="pf", name="pf")
                    for jj in range(4):
                        for m in range(2):
                            nc.tensor.transpose(
                                pf[:, 256 * jj + 128 * m:256 * jj + 128 * m + 128],
                                dn[:, m, 128 * jj:128 * jj + 128], ident[:])
                    j0 = 16 * q + jj0
                    kkf = j0 + 1
                    y0 = kkf + 7   # global pixel row of first output row
                    sc4 = _sub_ap(sc2_sb[:], 0, 128, kkf * C, [[1, 4 * C]])
                    # a2 = xn2*(sigma+1) + m   (== sc + xattn when not affine)
                    a1 = fin_p.tile([128, 1024], f32, tag="a1", name="a1")
                    nc.gpsimd.tensor_tensor(
                        out=a1[:], in0=sc4,
                        in1=_sub_ap(sg_sb[:], 0, 128, y0, [[1, 4], [0, C]]),
                        op=ALU.mult)
                    a2 = fin_p.tile([128, 1024], f32, tag="a2", name="a2")
                    nc.gpsimd.tensor_tensor(
                        out=a2[:], in0=a1[:],
                        in1=_sub_ap(m_sb[:], 0, 128, y0, [[1, 4], [0, C]]),
                        op=ALU.add)
                    if sc2_affine:
                        # a2 = xattn only; shortcut = xn2*g2 + b2 added here
                        a3 = fin_p.tile([128, 1024], f32, tag="a3", name="a3")
                        nc.gpsimd.tensor_tensor(
                            out=a3[:], in0=sc4,
                            in1=_sub_ap(g2r_sb[:], 0, 128, 0, [[0, 4], [1, C]]),
                            op=ALU.mult)
                        a4 = fin_p.tile([128, 1024], f32, tag="a4", name="a4")
                        nc.gpsimd.tensor_tensor(
                            out=a4[:], in0=a3[:],
                            in1=_sub_ap(b2r_sb[:], 0, 128, 0, [[0, 4], [1, C]]),
                            op=ALU.add)
                        a5 = fin_p.tile([128, 1024], f32, tag="a5", name="a5")
                        nc.vector.tensor_tensor(out=a5[:], in0=a2[:], in1=a4[:],
                                                op=ALU.add)
                        a2 = a5
                    out4 = fin_p.tile([128, 1024], f32, tag="out4", name="out4")
                    nc.vector.tensor_tensor(
                        out=out4[:], in0=a2[:], in1=pf[:], op=ALU.add)
                    nc.sync.dma_start(
                        out_d[j0 * WI:(j0 + 4) * WI, :]
                        .rearrange("(r p) c -> p r c", p=WI),
                        out4[:].rearrange("p (r c) -> p r c", r=4))

    nc.compile()
    return nc


def _prep(g1, b1, qkv_w, qkv_b, rpb_table, rel_idx, proj_w, g2, b2,
          up_w, up_b, dw_w, bn_g, bn_b, down_w, down_b):
    f = np.float32
    g1 = np.asarray(g1, f); b1 = np.asarray(b1, f)
    qkv_w = np.asarray(qkv_w, f); qkv_b = np.asarray(qkv_b, f)
    rpb = np.asarray(rpb_table, f); ridx = np.asarray(rel_idx)
    proj_w = np.asarray(proj_w, f)
    g2 = np.asarray(g2, f); b2 = np.asarray(b2, f)
    up_w = np.asarray(up_w, f); up_b = np.asarray(up_b, f)
    dw_w = np.asarray(dw_w, f); bn_g = np.asarray(bn_g, f)
    bn_b = np.asarray(bn_b, f)
    down_w = np.asarray(down_w, f); down_b = np.asarray(down_b, f)

    sc = HD ** -0.5
    wq = qkv_w[:C] * g1[None, :] * sc
    wk = qkv_w[C:2 * C] * g1[None, :]
    wv = qkv_w[2 * C:] * g1[None, :]
    bq = (qkv_b[:C] + qkv_w[:C] @ b1) * sc
    bk = qkv_b[C:2 * C] + qkv_w[C:2 * C] @ b1
    bv = qkv_b[2 * C:] + qkv_w[2 * C:] @ b1

    wqk = np.concatenate([wq, wk], 0).T.astype(BF16).copy()
    wv_t = wv.T.astype(BF16).copy()
    wproj = proj_w.T.astype(BF16).copy()

    bias = rpb[np.asarray(ridx).reshape(-1)].reshape(64, 64, NH).transpose(2, 0, 1)
    expb = np.zeros((128, 512), f)
    for h in range(NH):
        cc = 128 * (h % 4) + 64 * (h // 4)
        eb = np.exp(bias[h])
        expb[0:64, cc:cc + 64] = eb
        expb[64:128, cc:cc + 64] = eb
    expb = expb.astype(BF16)

    wup = (up_w * g2[None, :]).T.astype(BF16).copy()
    upb = (up_b + up_w @ b2).astype(f)
    bns = bn_g * (1.0 + BN_EPS) ** -0.5
    dww = dw_w.reshape(HID, 9) * bns[:, None]
    dwdiag = np.zeros((128, 9 * 8 * 32), f)
    pp = np.arange(128)
    for b in range(8):
        for t in range(9):
            dwdiag[pp, (t * 8 + b) * 32 + (pp % 32)] = dww[128 * b + pp, t]
    dwdiag = dwdiag.astype(BF16)
    # per-partition tap weights for the DVE/Pool MAC path: dwcol[p, 9b+t]
    dwcol = np.zeros((128, 8 * 9), f)
    for b in range(8):
        dwcol[:, 9 * b:9 * b + 9] = dww[128 * b:128 * (b + 1), :]
    # free-dim-replicated tap weights for the Pool tensor_tensor path
    dwfull = np.zeros((128, len(DW_POOL) * 9 * 512), f)
    for bp, b in enumerate(DW_POOL):
        for t in range(9):
            dwfull[:, (9 * bp + t) * 512:(9 * bp + t + 1) * 512] = \
                dww[128 * b:128 * (b + 1), t:t + 1]
    dwfull = dwfull.astype(BF16)
    wdn = down_w.T.astype(BF16).copy()

    def col_n(v, n):
        return np.asarray(v, f).reshape(n, 128).T.copy()

    qkb = col_n(np.concatenate([bq, bk]), 4)
    vbr = np.broadcast_to(bv[None, :], (128, C)).astype(f).copy()
    dnb = col_n(down_b, 2)
    g2r = np.broadcast_to(g2[None, :], (128, C)).astype(f).copy()
    b2r = np.broadcast_to(b2[None, :], (128, C)).astype(f).copy()

    flags = (bool(np.any(qkb)), bool(np.any(bv)), bool(np.any(down_b)),
             not (np.allclose(g2, 1.0) and np.allclose(b2, 0.0)))

    consts = dict(wqk=wqk, wv=wv_t, wproj=wproj, expb=expb, wup=wup, wdn=wdn,
                  dwdiag=dwdiag, dwcol=dwcol, dwfull=dwfull,
                  upb=col_n(upb, 8),
                  bnb=col_n(bn_b, 8), qkb=qkb,
                  vbr=vbr, dnb=dnb, g2r=g2r, b2r=b2r)
    return consts, flags


def kernel(x, H, W, g1, b1, qkv_w, qkv_b, rpb_table, rel_idx, proj_w,
           g2, b2, up_w, up_b, dw_w, bn_g, bn_b, down_w, down_b):
    global LAST_RESULTS
    from concourse.bass_utils import run_bass_kernel_spmd

    x = np.asarray(x, np.float32)
    consts, flags = _prep(g1, b1, qkv_w, qkv_b, rpb_table, rel_idx, proj_w,
                          g2, b2, up_w, up_b, dw_w, bn_g, bn_b, down_w, down_b)
    if flags not in _BUILD_CACHE:
        _BUILD_CACHE[flags] = _build(flags)
    nc = _BUILD_CACHE[flags]

    ximg = x.reshape(B_, HI, WI, C)
    in_maps = []
    for core in range(NCORES):
        b, top = core // 2, (core % 2 == 0)
        r0 = 0 if top else 64
        xs = np.zeros((AROWS, WI, C), np.float32)
        lo, hi = r0 - 8, r0 + 72
        slo, shi = max(lo, 0), min(hi, HI)
        xs[slo - lo:shi - lo] = ximg[b, slo:shi]
        m = {"xs": xs.reshape(TA, C)}
        m.update(consts)
        in_maps.append(m)

    res = run_bass_kernel_spmd(nc, in_maps, core_ids=list(range(NCORES)))
    LAST_RESULTS = res

    out = np.empty((B_, HI, WI, C), np.float32)
    for core in range(NCORES):
        b, top = core // 2, (core % 2 == 0)
        r0 = 0 if top else 64
        out[b, r0:r0 + 64] = res.results[core]["out"].reshape(OROWS, WI, C)
    return out.reshape(B_, HI * WI, C)
